# revision 1
# baseline (speedup 1.0000x reference)
"""GAT backbone (3-layer GATConv + graph pooling + loc-MLP) on 8 Trainium2
NeuronCores.

Strategy: dst-sharded edges. Each core owns a contiguous range of 6272
destination nodes (49 node-tiles of 128). Edges (with self-loops) are grouped
by dst node-tile on the host, split per node-tile into src<32768 / src>=32768
groups (dma_gather indices are int16), and padded to 128-edge tiles. Per
(layer, node-tile) the core gathers all K*128 table rows [h'|es|ed] (bf16,
512B rows) with TWO dma_gather custom-ucode calls (one per table half), loads
host-built fp8 one-hots (pdst: [dst,edge], pedge: [edge,dst]; zero columns
kill padded edge slots), broadcasts ed to edges with per-edge-tile one-hot
matmuls, computes w = exp(leaky(es+ed)) batched across the node-tile, scales
messages, and scatter-accumulates [w*h' | w] into PSUM with one-hot matmuls.
The normalized output is transformed (h_in @ [W|As_eff|Ad_eff]) into the next
layer's bf16 table, replicated across cores with an AllGather. Graph mean
pool is a per-shard matmul with a host-built one-hot, AllReduce-summed
across cores. The loc-MLP is computed redundantly on every core.
"""
import numpy as np

# ---------------------------------------------------------------------------
# runtime patch: this walrus build accepts at most ONE sync-wait command per
# instruction; Tile attaches several. Split extras into single-wait NOPs.
# ---------------------------------------------------------------------------
_PATCHED = [False]


def _install_tile_patch():
    if _PATCHED[0]:
        return
    import concourse.mybir as mybir
    from concourse.tile import TileContext
    from concourse.vector_clock import ScopedClock

    ctr = [0]

    def _split(insts):
        new = []
        for inst in insts:
            si = getattr(inst, "sync_info", None)
            try:
                ow = si.on_wait if si is not None else None
            except Exception:
                ow = None
            if ow is not None and len(ow) > 1:
                waits = list(ow)
                for w in waits[:-1]:
                    ctr[0] += 1
                    nop = mybir.InstNoOp(name=f"wsplit-{ctr[0]}", ins=[], outs=[])
                    nop.engine = inst.engine
                    nop.sync_info = mybir.SyncInfo(on_wait=[w], on_update=[])
                    new.append(nop)
                si.on_wait = waits[-1:]
            new.append(inst)
        insts[:] = new

    orig_lower = TileContext._lower_ordered_insts

    def patched_lower(self, ordered):
        for insts in ordered.values():
            _split(insts)
        return orig_lower(self, ordered)

    def patched_drain(self, tick_clock, wait_clock):
        drain_inst = self.nc.sync.drain()
        wait_clock.add_sem_waits(
            drain_inst.ins, ScopedClock({None: tick_clock.global_clock})
        )
        si = drain_inst.ins.sync_info
        if si is not None and si.on_wait and len(si.on_wait) > 1:
            waits = list(si.on_wait)
            si.on_wait = waits[:1]
            for w in waits[1:]:
                extra = self.nc.sync.drain()
                esi = extra.ins.sync_info
                if esi is None:
                    extra.ins.sync_info = mybir.SyncInfo(on_wait=[w], on_update=[])
                else:
                    esi.on_wait = [w]
        self.nc.all_engine_barrier()
        assert self.sems is not None
        popped = self.nc._tile_sem_poison_stack.pop()
        assert popped is self._sem_poison
        self.nc.clear_and_free_semaphores(list(self.sems.allocated().values()))
        self.nc.all_engine_barrier()

    TileContext._lower_ordered_insts = patched_lower
    TileContext._drain_and_barrier = patched_drain
    _PATCHED[0] = True


# ---------------------------------------------------------------------------
# problem constants (hardcoded per contract)
# ---------------------------------------------------------------------------
N_NODES = 50000
N_EDGES = 800000
N_GRAPHS = 64
N_LOCS = 50
HEADS = 3
OPH = 43                    # out per head
MID = HEADS * OPH           # 129
GH = 128                    # gat hidden (layer 2 out)
MLPH = 256
NEG = 0.2
N_CORES = 8
SHARD = 6272                # 49 * 128 dst nodes per core
NT = SHARD // 128           # 49 node-tiles
NPAD = SHARD * N_CORES      # 50176
HALF = 32768                # table split point for int16 gather indices
TW = 256                    # table row width (bf16, 512B rows)

_CACHE = {}


def _host_prep(x, loc, edge_index, batch, W0, as0, ad0, b0, W1, as1, ad1, b1,
               W2, as2, ad2, b2, Wl1, bl1, Wl2, bl2):
    import ml_dtypes
    f32 = np.float32
    bf16 = ml_dtypes.bfloat16
    fp8 = ml_dtypes.float8_e4m3

    src = np.concatenate([edge_index[0], np.arange(N_NODES, dtype=np.int64)])
    dst = np.concatenate([edge_index[1], np.arange(N_NODES, dtype=np.int64)])

    # group edges per (core, node-tile); within each, src<HALF first
    n_tiles_total = NPAD // 128              # 392
    key = (dst // 128) * 2 + (src >= HALF)
    order = np.argsort(key, kind="stable")
    src = src[order]
    dst = dst[order]
    counts2 = np.bincount(key[order], minlength=2 * n_tiles_total)
    bounds = np.zeros(2 * n_tiles_total + 1, np.int64)
    np.cumsum(counts2, out=bounds[1:])
    gstart = bounds[0::2]                    # [n_tiles_total + 1]
    asplit = bounds[1::2]                    # A/B split per tile

    nA = counts2[0::2].reshape(N_CORES, NT)
    nB = counts2[1::2].reshape(N_CORES, NT)
    KA_nt = tuple(int(np.ceil(nA[:, t].max() / 128)) for t in range(NT))
    KB_nt = tuple(int(np.ceil(nB[:, t].max() / 128)) for t in range(NT))
    K = max(a + b for a, b in zip(KA_nt, KB_nt))

    # per (core, nt): padded edge list (idx into half-table, rel dst, valid)
    idx_flat = np.zeros((N_CORES, NT, K * 128), np.int16)
    rel_flat = np.zeros((N_CORES, NT, K * 128), np.int64)
    val_flat = np.zeros((N_CORES, NT, K * 128), bool)
    for g in range(n_tiles_total):
        c, t = divmod(g, NT)
        s, m, e = gstart[g], asplit[g], gstart[g + 1]
        na, nb = m - s, e - m
        ka = KA_nt[t]
        idx_flat[c, t, :na] = src[s:m].astype(np.int16)
        rel_flat[c, t, :na] = dst[s:m] - g * 128
        val_flat[c, t, :na] = True
        o = ka * 128
        idx_flat[c, t, o:o + nb] = (src[m:e] - HALF).astype(np.int16)
        rel_flat[c, t, o:o + nb] = dst[m:e] - g * 128
        val_flat[c, t, o:o + nb] = True

    # wrap idx into 16 partitions, replicate x8: [C, 128, NT, K*8]
    iw = idx_flat.reshape(N_CORES, NT, K * 8, 16).transpose(0, 3, 1, 2)
    idx_dev = np.ascontiguousarray(np.tile(iw, (1, 8, 1, 1)))

    rel_i = rel_flat.reshape(N_CORES, NT, K, 128)         # [C, NT, K, 128e]
    val_i = val_flat.reshape(N_CORES, NT, K, 128)
    eye8 = np.eye(128, dtype=fp8)
    oh = eye8[rel_i]                                      # [C, NT, K, 128e, 128d]
    oh[~val_i] = 0
    # pdst[d, e] one-hot per (nt,k): [C, NT, 128d, K*128e]
    pdst_dev = np.ascontiguousarray(
        oh.transpose(0, 1, 4, 2, 3)).reshape(N_CORES, NT, 128, K * 128)
    # pedge[e, d] one-hot per (nt,k): [C, NT, 128e, K*128d]
    pedge_dev = np.ascontiguousarray(
        oh.transpose(0, 1, 3, 2, 4)).reshape(N_CORES, NT, 128, K * 128)

    # pooling one-hot per core [128, NT, 64] and counts
    node = np.arange(NPAD)
    valid = node < N_NODES
    gid = np.where(valid, batch[np.minimum(node, N_NODES - 1)], 0)
    onehot = np.zeros((NPAD, N_GRAPHS), f32)
    onehot[valid, gid[valid]] = 1.0
    bp = onehot.reshape(N_CORES, NT, 128, N_GRAPHS).transpose(0, 2, 1, 3)
    bp = np.ascontiguousarray(bp).astype(bf16)            # [C, 128, NT, 64]
    cnt = np.bincount(batch, minlength=N_GRAPHS).astype(f32)
    cntinv = (1.0 / np.maximum(cnt, 1.0)).reshape(N_GRAPHS, 1).astype(f32)

    # weight packing: rhs_l = [W_l | W_l@Amat_s | W_l@Amat_d]
    def amat(a):
        h, o = a.shape
        m = np.zeros((h * o, h), f32)
        for i in range(h):
            m[i * o:(i + 1) * o, i] = a[i]
        return m

    rhs0 = np.concatenate([W0, W0 @ amat(as0), W0 @ amat(ad0)], axis=1).astype(bf16)
    rhs1 = np.concatenate([W1, W1 @ amat(as1), W1 @ amat(ad1)], axis=1).astype(bf16)
    rhs2 = np.concatenate([W2, W2 @ amat(as2), W2 @ amat(ad2)], axis=1).astype(bf16)

    xT = np.zeros((6, NPAD), bf16)
    xT[:, :N_NODES] = np.asarray(x, f32).T.astype(bf16)
    xT_own = np.ascontiguousarray(
        xT.reshape(6, N_CORES, SHARD).transpose(1, 0, 2))

    bb0 = np.tile(np.asarray(b0, f32)[None, :], (128, 1)).astype(bf16)
    bb1 = np.tile(np.asarray(b1, f32)[None, :], (128, 1)).astype(bf16)
    bb2 = np.tile(np.asarray(b2, f32)[None, :], (128, 1)).astype(bf16)

    locT = np.asarray(loc, f32).reshape(N_GRAPHS * N_LOCS, 2).T.copy()
    ident = np.eye(128, dtype=bf16)

    common = dict(
        xT=xT, rhs0=rhs0,
        rhs1a=rhs1[:128].copy(), rhs1b=rhs1[128:].copy(),
        rhs2a=rhs2[:128].copy(), rhs2b=rhs2[128:].copy(),
        bb0=bb0, bb1=bb1, bb2=bb2, cntinv=cntinv,
        locT=locT, wl1=np.asarray(Wl1, f32),
        bl1c=np.asarray(bl1, f32).reshape(-1, 1),
        wl2=np.asarray(Wl2, f32), bl2c=np.asarray(bl2, f32).reshape(-1, 1),
        ident=ident,
    )
    in_maps = []
    for c in range(N_CORES):
        m = dict(common)
        m["idx"] = idx_dev[c]
        m["bpool"] = bp[c]
        m["xTo"] = xT_own[c]
        m["pdst"] = pdst_dev[c]
        m["pedge"] = pedge_dev[c]
        in_maps.append(m)
    return in_maps, (K, KA_nt, KB_nt)


def _build(Kinfo):
    K, KA_nt, KB_nt = Kinfo
    _install_tile_patch()
    import concourse.bass as bass
    import concourse.mybir as mybir
    import concourse.tile as tile
    from concourse import library_config
    from concourse.library_overlay import lower_extended_insts

    f32 = mybir.dt.float32
    bf16 = mybir.dt.bfloat16
    fp8 = mybir.dt.float8e4
    i16 = mybir.dt.int16
    AF = mybir.ActivationFunctionType

    nc = bass.Bass(num_devices=N_CORES)

    inp = {}
    for name, shape, dt in [
        ("xT", [6, NPAD], bf16), ("xTo", [6, SHARD], bf16),
        ("idx", [128, NT, K * 8], i16),
        ("pdst", [NT, 128, K * 128], fp8),
        ("pedge", [NT, 128, K * 128], fp8),
        ("bpool", [128, NT, N_GRAPHS], bf16), ("cntinv", [N_GRAPHS, 1], f32),
        ("rhs0", [6, MID + 6], bf16),
        ("rhs1a", [128, MID + 6], bf16), ("rhs1b", [1, MID + 6], bf16),
        ("rhs2a", [128, GH + 2], bf16), ("rhs2b", [1, GH + 2], bf16),
        ("bb0", [128, MID], bf16), ("bb1", [128, MID], bf16),
        ("bb2", [128, GH], bf16),
        ("locT", [2, N_GRAPHS * N_LOCS], f32), ("wl1", [2, MLPH], f32),
        ("bl1c", [MLPH, 1], f32), ("wl2", [MLPH, GH], f32), ("bl2c", [GH, 1], f32),
        ("ident", [128, 128], bf16),
    ]:
        inp[name] = nc.dram_tensor(name, shape, dt, kind="ExternalInput")

    out = nc.dram_tensor("out", [N_GRAPHS, GH * 2], f32, kind="ExternalOutput")

    # tables (bf16, 512B rows)
    T = [
        nc.dram_tensor("T0", [NPAD, TW], bf16, kind="Internal"),
        nc.dram_tensor("T1", [NPAD, TW], bf16, kind="Internal",
                       addr_space="Shared"),
        nc.dram_tensor("T2", [NPAD, TW], bf16, kind="Internal",
                       addr_space="Shared"),
    ]
    Tsh = [
        nc.dram_tensor("Tsh1", [SHARD, TW], bf16, kind="Internal"),
        nc.dram_tensor("Tsh2", [SHARD, TW], bf16, kind="Internal"),
    ]
    s_in = nc.dram_tensor("s_in", [N_GRAPHS, GH], f32, kind="Internal")
    s_out = nc.dram_tensor("s_out", [N_GRAPHS, GH], f32, kind="Internal",
                           addr_space="Shared")

    # per-layer config: (feat width F, heads H)
    LCFG = [(MID, HEADS), (MID, HEADS), (GH, 1)]
    RG = [list(range(N_CORES))]

    with tile.TileContext(nc) as tc:
        with tc.tile_pool(name="const", bufs=1) as cp, \
             tc.tile_pool(name="sb", bufs=4) as sb, \
             tc.tile_pool(name="gp", bufs=2) as gp, \
             tc.tile_pool(name="ed", bufs=1) as edp, \
             tc.tile_pool(name="mlp", bufs=1) as mlppool, \
             tc.tile_pool(name="ps", bufs=3, space="PSUM") as ps, \
             tc.tile_pool(name="pse", bufs=2, space="PSUM") as pse, \
             tc.tile_pool(name="psacc", bufs=2, space="PSUM") as psacc, \
             tc.tile_pool(name="pspool", bufs=1, space="PSUM") as pspool:

            nc.gpsimd.load_library(library_config.mlp)

            # one register per distinct gather count (to_reg pool is small);
            # dma_gather ucode caps at 1024 indices per call -> chunk to <=8
            # tiles per call
            GMAX = 8
            sizes = set()
            for v in set(KA_nt) | set(KB_nt):
                while v > 0:
                    sizes.add(min(v, GMAX) * 128)
                    v -= min(v, GMAX)
            nidx_regs = {n: nc.gpsimd.to_reg(n) for n in sorted(sizes)}

            ident = cp.tile([128, 128], bf16)
            nc.sync.dma_start(ident[:], inp["ident"][:, :])

            # constant weights / biases
            t0r = cp.tile([6, MID + 6], bf16, tag="rhs0")
            nc.sync.dma_start(t0r[:], inp["rhs0"][:, :])
            rhs_a, rhs_b = {}, {}
            for l, nma, nmb in [(1, "rhs1a", "rhs1b"), (2, "rhs2a", "rhs2b")]:
                wdt = inp[nma].shape[1]
                ta = cp.tile([128, wdt], bf16, tag=f"rhsa{l}")
                nc.sync.dma_start(ta[:], inp[nma][:, :])
                tb = cp.tile([1, wdt], bf16, tag=f"rhsb{l}")
                nc.sync.dma_start(tb[:], inp[nmb][:, :])
                rhs_a[l], rhs_b[l] = ta, tb
            bb_sb = []
            for l, nm in enumerate(["bb0", "bb1", "bb2"]):
                t = cp.tile(list(inp[nm].shape), bf16, tag=f"bb{l}")
                nc.sync.dma_start(t[:], inp[nm][:, :])
                bb_sb.append(t)

            # preloaded gather indices and pooling one-hot
            idx_sb = cp.tile([128, NT, K * 8], i16, tag="idxall")
            nc.sync.dma_start(idx_sb[:], inp["idx"][:, :, :])
            bp_sb = cp.tile([128, NT, N_GRAPHS], bf16, tag="bpall")
            nc.sync.dma_start(bp_sb[:], inp["bpool"][:, :, :])

            # ed values for own shard, per layer: [128, NT, H]
            ed_all = [edp.tile([128, NT, 3], bf16, tag=f"edall{l}",
                               name=f"edall{l}") for l in range(3)]

            # ---------------- stage A: build T0 for all nodes ----------------
            # 8 node-tiles per chunk: 1 input DMA + 8 matmuls + 1 output DMA
            for j8 in range(NPAD // 1024):
                xs = sb.tile([6, 1024], bf16, tag="xs")
                nc.sync.dma_start(xs[:], inp["xT"][:, j8 * 1024:(j8 + 1) * 1024])
                stg8 = sb.tile([128, 8, TW], bf16, tag="stg8")
                nc.vector.memset(stg8[:, :, MID + 6:], 0.0)
                for jj in range(8):
                    ptab = ps.tile([128, MID + 6], f32, space="PSUM", tag="pscr")
                    nc.tensor.matmul(ptab[:], lhsT=xs[:, jj * 128:(jj + 1) * 128],
                                     rhs=t0r[:], start=True, stop=True)
                    nc.vector.tensor_copy(stg8[:, jj, :MID + 6], ptab[:])
                nc.sync.dma_start(
                    T[0][j8 * 1024:(j8 + 1) * 1024, :].rearrange(
                        "(t p) c -> p t c", p=128),
                    stg8[:])
            # ed0 for own shard (from per-core xTo input)
            xo = cp.tile([6, SHARD], bf16, tag="xo")
            nc.sync.dma_start(xo[:], inp["xTo"][:, :])
            for t in range(NT):
                pe0 = ps.tile([128, 6], f32, space="PSUM", tag="pscr")
                nc.tensor.matmul(pe0[:], lhsT=xo[:, t * 128:(t + 1) * 128],
                                 rhs=t0r[:, MID:MID + 6], start=True, stop=True)
                nc.vector.tensor_copy(ed_all[0][:, t, :], pe0[:, 3:6])

            # ---------------- loc MLP (independent) ----------------
            locT = mlppool.tile([2, N_GRAPHS * N_LOCS], f32)
            nc.sync.dma_start(locT[:], inp["locT"][:, :])
            wl1 = mlppool.tile([2, MLPH], f32)
            nc.sync.dma_start(wl1[:], inp["wl1"][:, :])
            bl1c = mlppool.tile([128, 2], f32)
            nc.sync.dma_start(bl1c[:], inp["bl1c"][:, 0:1].rearrange(
                "(h p) o -> p (h o)", p=128))
            wl2a = mlppool.tile([128, GH], f32)
            nc.sync.dma_start(wl2a[:], inp["wl2"][0:128, :])
            wl2b = mlppool.tile([128, GH], f32)
            nc.sync.dma_start(wl2b[:], inp["wl2"][128:256, :])
            bl2c = mlppool.tile([GH, 1], f32)
            nc.sync.dma_start(bl2c[:], inp["bl2c"][:, :])
            identf = mlppool.tile([128, 128], f32)
            nc.vector.tensor_copy(identf[:], ident[:])
            mid_sb = [mlppool.tile([128, N_GRAPHS * N_LOCS], f32,
                                   tag=f"mid{h}", name=f"mid{h}")
                      for h in range(2)]
            CH = 400
            nch = (N_GRAPHS * N_LOCS) // CH
            for h in range(2):
                for c in range(nch):
                    pm = ps.tile([128, CH], f32, space="PSUM", tag="pscr")
                    nc.tensor.matmul(pm[:], lhsT=wl1[:, h * 128:(h + 1) * 128],
                                     rhs=locT[:, c * CH:(c + 1) * CH],
                                     start=True, stop=True)
                    nc.scalar.activation(mid_sb[h][:, c * CH:(c + 1) * CH], pm[:],
                                         AF.Tanh, bias=bl1c[:, h:h + 1])
            lpT = mlppool.tile([128, N_GRAPHS], f32)
            for c in range(nch):
                po = ps.tile([128, CH], f32, space="PSUM", tag="pscr")
                nc.tensor.matmul(po[:], lhsT=wl2a[:],
                                 rhs=mid_sb[0][:, c * CH:(c + 1) * CH],
                                 start=True, stop=False)
                nc.tensor.matmul(po[:], lhsT=wl2b[:],
                                 rhs=mid_sb[1][:, c * CH:(c + 1) * CH],
                                 start=False, stop=True)
                ng = CH // N_LOCS
                nc.vector.reduce_sum(
                    lpT[:, c * ng:(c + 1) * ng],
                    po[:].rearrange("p (g l) -> p g l", l=N_LOCS),
                    axis=mybir.AxisListType.X)
            lpT2 = mlppool.tile([128, N_GRAPHS], f32)
            nc.vector.tensor_scalar(lpT2[:], lpT[:], 1.0 / N_LOCS, bl2c[:],
                                    mybir.AluOpType.mult, mybir.AluOpType.add)
            plp = ps.tile([N_GRAPHS, 128], f32, space="PSUM", tag="pscr")
            nc.tensor.matmul(plp[:], lhsT=lpT2[:], rhs=identf[:],
                             start=True, stop=True)
            out_sb = mlppool.tile([N_GRAPHS, 2 * GH], f32)
            nc.vector.tensor_copy(out_sb[:, GH:], plp[:])

            # ---------------- GAT layers ----------------
            psum_S = pspool.tile([N_GRAPHS, GH], f32, space="PSUM")

            for l in range(3):
                F, H = LCFG[l]
                RW = F + H                       # scatter width
                O = F // H
                tab = T[l]
                for nt in range(NT):
                    KA, KB = KA_nt[nt], KB_nt[nt]
                    Kt = KA + KB
                    g = gp.tile([128, K, TW], bf16, tag="g")
                    for base, kt, tap in [(0, KA, tab[0:HALF, :]),
                                          (KA, KB, tab[HALF:NPAD, :])]:
                        done = 0
                        while done < kt:
                            m = min(GMAX, kt - done)
                            t0 = base + done
                            nc.gpsimd.dma_gather(
                                out_ap=g[:, t0:t0 + m, :], in_ap=tap,
                                idxs_ap=idx_sb[:, nt, t0 * 8:(t0 + m) * 8],
                                num_idxs=m * 128,
                                num_idxs_reg=nidx_regs[m * 128],
                                elem_size=TW)
                            done += m
                    pd = gp.tile([128, K * 128], fp8, tag="pd")
                    nc.sync.dma_start(pd[:, 0:Kt * 128],
                                      inp["pdst"][nt, :, 0:Kt * 128])
                    pe_ = gp.tile([128, K * 128], fp8, tag="pe")
                    nc.scalar.dma_start(pe_[:, 0:Kt * 128],
                                        inp["pedge"][nt, :, 0:Kt * 128])
                    # ed broadcast to edges: one-hot matmuls per edge-tile
                    pede = pse.tile([128, K, 3], f32, space="PSUM", tag="pede")
                    for t in range(Kt):
                        nc.tensor.matmul(pede[:, t, :H],
                                         lhsT=pd[:, t * 128:(t + 1) * 128],
                                         rhs=ed_all[l][:, nt, :H],
                                         start=True, stop=True)
                    # w = exp(leaky(es + ed)) for all K tiles at once
                    esum = sb.tile([128, K, 3], f32, tag="esum")
                    nc.vector.tensor_tensor(
                        out=esum[:, 0:Kt, :H], in0=g[:, 0:Kt, F:F + H],
                        in1=pede[:, 0:Kt, :H], op=mybir.AluOpType.add)
                    lk = sb.tile([128, K, 3], f32, tag="lk")
                    nc.scalar.activation(lk[:, 0:Kt, :H], esum[:, 0:Kt, :H],
                                         AF.Prelu, alpha=NEG)
                    w = sb.tile([128, K, 3], bf16, tag="w")
                    nc.scalar.activation(w[:, 0:Kt, :H], lk[:, 0:Kt, :H], AF.Exp)
                    # rhs_t = [w*h' | w]
                    rt = gp.tile([128, K, RW], bf16, tag="rt")
                    for h in range(H):
                        nc.vector.tensor_tensor(
                            out=rt[:, 0:Kt, h * O:(h + 1) * O],
                            in0=g[:, 0:Kt, h * O:(h + 1) * O],
                            in1=w[:, 0:Kt, h:h + 1].to_broadcast([128, Kt, O]),
                            op=mybir.AluOpType.mult)
                    nc.vector.tensor_copy(rt[:, 0:Kt, F:F + H], w[:, 0:Kt, :H])
                    # scatter-accumulate into PSUM
                    acc = psacc.tile([128, RW], f32, space="PSUM", tag="acc")
                    for t in range(Kt):
                        nc.tensor.matmul(acc[:],
                                         lhsT=pe_[:, t * 128:(t + 1) * 128],
                                         rhs=rt[:, t, :],
                                         start=(t == 0), stop=(t == Kt - 1))
                    # epilogue for this node-tile
                    zc = sb.tile([128, 3], f32, tag="zc")
                    nc.vector.tensor_scalar_max(zc[:, :H], acc[:, F:F + H], 1e-30)
                    zr = sb.tile([128, 3], f32, tag="zr")
                    nc.vector.reciprocal(zr[:, :H], zc[:, :H])
                    # normalize: u_h = acc_h * zr_h  (Copy with per-row scale)
                    ob = sb.tile([128, F], bf16, tag="ob")
                    for h in range(H):
                        nc.scalar.activation(ob[:, h * O:(h + 1) * O],
                                             acc[:, h * O:(h + 1) * O],
                                             AF.Copy, scale=zr[:, h:h + 1])
                    if l < 2:
                        hin = sb.tile([128, F], bf16, tag="hin")
                        nc.vector.tensor_add(hin[:], ob[:], bb_sb[l][:, :F])
                        nc.scalar.activation(hin[:], hin[:], AF.Prelu, alpha=NEG)
                        # transpose hin -> [F, 128] in two pieces
                        ph1 = ps.tile([128, 128], f32, space="PSUM", tag="pscr")
                        nc.tensor.matmul(ph1[:], lhsT=hin[:, 0:128], rhs=ident[:],
                                         start=True, stop=True)
                        hTa = sb.tile([128, 128], bf16, tag="hTa")
                        nc.vector.tensor_copy(hTa[:], ph1[:])
                        ph2 = ps.tile([1, 128], f32, space="PSUM", tag="pscr")
                        nc.tensor.matmul(ph2[:], lhsT=hin[:, 128:129], rhs=ident[:],
                                         start=True, stop=True)
                        hTb = sb.tile([1, 128], bf16, tag="hTb")
                        nc.vector.tensor_copy(hTb[:], ph2[:])
                        nF2, nH2 = LCFG[l + 1]
                        ptab = ps.tile([128, nF2 + 2 * nH2], f32, space="PSUM",
                                       tag="pscr")
                        nc.tensor.matmul(ptab[:], lhsT=hTa[:],
                                         rhs=rhs_a[l + 1][:],
                                         start=True, stop=False)
                        nc.tensor.matmul(ptab[:], lhsT=hTb[:],
                                         rhs=rhs_b[l + 1][:],
                                         start=False, stop=True)
                        stg = sb.tile([128, TW], bf16, tag="stg")
                        nc.vector.memset(stg[:, nF2 + 2 * nH2:], 0.0)
                        nc.vector.tensor_copy(stg[:, :nF2 + 2 * nH2], ptab[:])
                        nc.sync.dma_start(
                            Tsh[l][nt * 128:(nt + 1) * 128, :], stg[:])
                        nc.vector.tensor_copy(
                            ed_all[l + 1][:, nt, :nH2],
                            ptab[:, nF2 + nH2:nF2 + 2 * nH2])
                    else:
                        ob2 = sb.tile([128, F], bf16, tag="ob2")
                        nc.vector.tensor_add(ob2[:], ob[:], bb_sb[l][:, :F])
                        nc.tensor.matmul(psum_S[:], lhsT=bp_sb[:, nt, :],
                                         rhs=ob2[:],
                                         start=(nt == 0), stop=(nt == NT - 1))
                if l < 2:
                    nc.gpsimd.collective_compute(
                        "AllGather", mybir.AluOpType.bypass, replica_groups=RG,
                        ins=[Tsh[l][:, :]], outs=[T[l + 1][0:NPAD, :]])

            # pooling: AllReduce of per-shard sums, then divide by counts
            ssb = sb.tile([N_GRAPHS, GH], f32, tag="ssb")
            nc.vector.tensor_copy(ssb[:], psum_S[:])
            nc.sync.dma_start(s_in[:, :], ssb[:])
            nc.gpsimd.collective_compute(
                "AllReduce", mybir.AluOpType.add, replica_groups=RG,
                ins=[s_in[:, :]], outs=[s_out[:, :]])
            sfull = sb.tile([N_GRAPHS, GH], f32, tag="sfull")
            nc.sync.dma_start(sfull[:], s_out[:, :])
            civ = sb.tile([N_GRAPHS, 1], f32, tag="civ")
            nc.sync.dma_start(civ[:], inp["cntinv"][:, :])
            nc.vector.tensor_scalar_mul(out_sb[:, 0:GH], sfull[:], civ[:])
            nc.sync.dma_start(out[:, :], out_sb[:])

    lower_extended_insts(nc)
    return nc


def kernel(**inputs):
    key = "k"
    in_maps, Kinfo = _host_prep(**inputs)
    if key not in _CACHE or _CACHE[key][1] != Kinfo:
        nc = _build(Kinfo)
        _CACHE[key] = (nc, Kinfo)
    nc = _CACHE[key][0]
    from concourse.bass_utils import run_bass_kernel_spmd
    res = run_bass_kernel_spmd(nc, in_maps, core_ids=list(range(N_CORES)))
    return np.asarray(res.results[0]["out"])



# revision 8
# speedup vs baseline: 1.4552x; 1.4552x over previous
"""GAT backbone (3-layer GATConv + graph pooling + loc-MLP) on 8 Trainium2
NeuronCores.

Strategy (v2): dst-sharded edges, gather-minimized.
- Layer 0 messages are rank-6 in x: the host pre-expands [x[src]|es0] per
  edge slot (sequential DMA, no gather), the scatter runs at width 21
  ([w_h*x | w]), and a per-tile Wexp matmul recovers the 129-wide GAT0
  output. No T0 table, no layer-0 dma_gather.
- Layers 1/2 gather 256-byte table rows [fp16 scale | fp16 es | int8 h']
  via dma_gather in large tile-packed calls (~4.6K indices each, enlarged
  SWDGE descriptor ring) from two half-tables T_a/T_b. Node positions are
  permuted so half A = all cores' first 24 dst-tiles: each half's
  AllGather unblocks its gather calls independently.
- Self-loops are excluded from edge lists and added analytically in each
  tile epilogue (own-shard rows + es/ed caches stay in SBUF).
- Per-edge attention: one-hot fp8 matmuls broadcast ed to edges (pdst) and
  scatter [w*h'|w] into PSUM (pedge), as in v1.
"""
import numpy as np

# ---------------------------------------------------------------------------
# runtime patch: this walrus build accepts at most ONE sync-wait command per
# instruction; Tile attaches several. Split extras into single-wait NOPs.
# ---------------------------------------------------------------------------
_PATCHED = [False]


def _install_tile_patch():
    if _PATCHED[0]:
        return
    import concourse.mybir as mybir
    from concourse.tile import TileContext
    from concourse.vector_clock import ScopedClock

    ctr = [0]

    def _split(insts):
        new = []
        for inst in insts:
            si = getattr(inst, "sync_info", None)
            try:
                ow = si.on_wait if si is not None else None
            except Exception:
                ow = None
            if ow is not None and len(ow) > 1:
                waits = list(ow)
                for w in waits[:-1]:
                    ctr[0] += 1
                    nop = mybir.InstNoOp(name=f"wsplit-{ctr[0]}", ins=[], outs=[])
                    nop.engine = inst.engine
                    nop.sync_info = mybir.SyncInfo(on_wait=[w], on_update=[])
                    new.append(nop)
                si.on_wait = waits[-1:]
            new.append(inst)
        insts[:] = new

    orig_lower = TileContext._lower_ordered_insts

    def patched_lower(self, ordered):
        for insts in ordered.values():
            _split(insts)
        return orig_lower(self, ordered)

    def patched_drain(self, tick_clock, wait_clock):
        drain_inst = self.nc.sync.drain()
        wait_clock.add_sem_waits(
            drain_inst.ins, ScopedClock({None: tick_clock.global_clock})
        )
        si = drain_inst.ins.sync_info
        if si is not None and si.on_wait and len(si.on_wait) > 1:
            waits = list(si.on_wait)
            si.on_wait = waits[:1]
            for w in waits[1:]:
                extra = self.nc.sync.drain()
                esi = extra.ins.sync_info
                if esi is None:
                    extra.ins.sync_info = mybir.SyncInfo(on_wait=[w], on_update=[])
                else:
                    esi.on_wait = [w]
        self.nc.all_engine_barrier()
        assert self.sems is not None
        popped = self.nc._tile_sem_poison_stack.pop()
        assert popped is self._sem_poison
        self.nc.clear_and_free_semaphores(list(self.sems.allocated().values()))
        self.nc.all_engine_barrier()

    TileContext._lower_ordered_insts = patched_lower
    TileContext._drain_and_barrier = patched_drain
    _PATCHED[0] = True


# ---------------------------------------------------------------------------
# problem constants (hardcoded per contract)
# ---------------------------------------------------------------------------
N_NODES = 50000
N_EDGES = 800000
N_GRAPHS = 64
N_LOCS = 50
HEADS = 3
OPH = 43                    # out per head
MID = HEADS * OPH           # 129
GH = 128                    # gat hidden (layer 2 out)
MLPH = 256
NEG = 0.2
N_CORES = 8
SHARD = 6272                # 49 * 128 dst nodes per core
NT = SHARD // 128           # 49 node-tiles
NPAD = SHARD * N_CORES      # 50176
NT_A = 24                   # tiles per core in half A
NT_B = NT - NT_A            # 25
HA = NT_A * 128             # 3072 rows/core in half A
HB = NT_B * 128             # 3200
A_TOT = HA * N_CORES        # 24576 rows in table A
B_TOT = HB * N_CORES        # 25600 rows in table B
ROWB = 256                  # table row bytes
CALL_COLS = 36              # max 128-edge columns per dma_gather call
DMA_SCRATCH = 40960         # SWDGE descriptor ring bytes/partition

_CACHE = {}


def _pi(n):
    """table position of node n: half-A shards first, then half-B shards."""
    c, r = n // SHARD, n % SHARD
    return np.where(r < HA, c * HA + r, A_TOT + c * HB + (r - HA))


def _host_prep(x, loc, edge_index, batch, W0, as0, ad0, b0, W1, as1, ad1, b1,
               W2, as2, ad2, b2, Wl1, bl1, Wl2, bl2):
    import ml_dtypes
    f32 = np.float32
    bf16 = ml_dtypes.bfloat16
    fp8 = ml_dtypes.float8_e4m3

    def amat(a):
        h, o = a.shape
        m = np.zeros((h * o, h), f32)
        for i in range(h):
            m[i * o:(i + 1) * o, i] = a[i]
        return m

    x = np.asarray(x, f32)
    src = np.asarray(edge_index[0])
    dst = np.asarray(edge_index[1])
    p_src = _pi(src)

    core = dst // SHARD
    tile = (dst % SHARD) // 128
    isB = (p_src >= A_TOT).astype(np.int64)
    key = (core * NT + tile) * 2 + isB
    order = np.argsort(key, kind="stable")
    src_s, dst_s, psrc_s = src[order], dst[order], p_src[order]
    key_s = key[order]
    cnt = np.bincount(key_s, minlength=N_CORES * NT * 2)
    bounds = np.zeros(N_CORES * NT * 2 + 1, np.int64)
    np.cumsum(cnt, out=bounds[1:])
    nA = cnt[0::2].reshape(N_CORES, NT)
    nB = cnt[1::2].reshape(N_CORES, NT)
    KA = tuple(int(np.ceil(nA[:, t].max() / 128)) for t in range(NT))
    KB = tuple(int(np.ceil(nB[:, t].max() / 128)) for t in range(NT))
    offA = np.concatenate([[0], np.cumsum(KA)]).astype(int)   # [NT+1]
    offB = np.concatenate([[0], np.cumsum(KB)]).astype(int)
    CA, CB = int(offA[-1]), int(offB[-1])
    TOT = sum(KA[t] + KB[t] for t in range(NT))
    offT = np.concatenate([[0], np.cumsum([KA[t] + KB[t] for t in range(NT)])]).astype(int)

    # per-edge slot assignment -> idx (A/B spaces), rel-dst one-hots, xe
    idxA = np.zeros((N_CORES, CA * 128), np.int16)
    idxB = np.zeros((N_CORES, CB * 128), np.int16)
    rel = np.zeros((N_CORES, TOT * 128), np.int64)
    val = np.zeros((N_CORES, TOT * 128), bool)
    xe = np.zeros((N_CORES, TOT * 128, 12), f32)

    es0w = W0 @ amat(as0)                       # [6,3]
    ed0w = W0 @ amat(ad0)
    es0_node = x @ es0w                         # [N,3]
    ed0_node = x @ ed0w

    for c in range(N_CORES):
        for t in range(NT):
            g = (c * NT + t) * 2
            sA, eA = bounds[g], bounds[g + 1]
            sB, eB = bounds[g + 1], bounds[g + 2]
            na, nb = eA - sA, eB - sB
            # A slots
            a0 = offA[t] * 128
            idxA[c, a0:a0 + na] = psrc_s[sA:eA].astype(np.int16)
            fa = offT[t] * 128
            rel[c, fa:fa + na] = dst_s[sA:eA] % 128
            val[c, fa:fa + na] = True
            xe[c, fa:fa + na, 0:6] = x[src_s[sA:eA]]
            xe[c, fa:fa + na, 6:9] = es0_node[src_s[sA:eA]]
            # B slots
            b0_ = offB[t] * 128
            idxB[c, b0_:b0_ + nb] = (psrc_s[sB:eB] - A_TOT).astype(np.int16)
            fb = (offT[t] + KA[t]) * 128
            rel[c, fb:fb + nb] = dst_s[sB:eB] % 128
            val[c, fb:fb + nb] = True
            xe[c, fb:fb + nb, 0:6] = x[src_s[sB:eB]]
            xe[c, fb:fb + nb, 6:9] = es0_node[src_s[sB:eB]]

    # idx wrap: [C, 128, CA*8] (16-partition wrap replicated x8)
    def wrap_idx(a, C):
        iw = a.reshape(N_CORES, C * 8, 16).transpose(0, 2, 1)
        return np.ascontiguousarray(np.tile(iw, (1, 8, 1)))
    idxA_dev = wrap_idx(idxA, CA)
    idxB_dev = wrap_idx(idxB, CB)

    # one-hots: pdst [d, e], pedge [e, d] per 128-edge column
    rel_i = rel.reshape(N_CORES, TOT, 128)
    val_i = val.reshape(N_CORES, TOT, 128)
    eye8 = np.eye(128, dtype=fp8)
    oh = eye8[rel_i]                            # [C, TOT, 128e, 128d]
    oh[~val_i] = 0
    pdst_dev = np.ascontiguousarray(
        oh.transpose(0, 3, 1, 2)).reshape(N_CORES, 128, TOT * 128)
    pedge_dev = np.ascontiguousarray(
        oh.transpose(0, 2, 1, 3)).reshape(N_CORES, 128, TOT * 128)

    xe_dev = np.ascontiguousarray(
        xe.reshape(N_CORES, TOT, 128, 12).transpose(0, 2, 1, 3)).astype(bf16)

    # own-shard per-node data for self-loop terms (layer 0); pad nodes -> 0
    xpad = np.zeros((NPAD, 6), f32)
    xpad[:N_NODES] = x
    es0pad = np.zeros((NPAD, 3), f32)
    es0pad[:N_NODES] = es0_node
    ed0pad = np.zeros((NPAD, 3), f32)
    ed0pad[:N_NODES] = ed0_node
    own = np.arange(SHARD)
    x_own = np.zeros((N_CORES, 128, NT, 6), f32)
    es0_own = np.zeros((N_CORES, 128, NT, 3), f32)
    ed0_own = np.zeros((N_CORES, 128, NT, 3), f32)
    for c in range(N_CORES):
        nodes = c * SHARD + own                  # global dst ids
        x_own[c] = xpad[nodes].reshape(NT, 128, 6).transpose(1, 0, 2)
        es0_own[c] = es0pad[nodes].reshape(NT, 128, 3).transpose(1, 0, 2)
        ed0_own[c] = ed0pad[nodes].reshape(NT, 128, 3).transpose(1, 0, 2)

    # gather call packing: greedy tiles while sum K <= CALL_COLS
    def pack_calls(Ks, offs):
        calls = []   # (colstart, ncols, first_tile, ntiles)
        t = 0
        while t < NT:
            t0, c0 = t, offs[t]
            while t < NT and offs[t + 1] - c0 <= CALL_COLS:
                t += 1
            calls.append((int(c0), int(offs[t] - c0), t0, t - t0))
        return calls
    callsA = pack_calls(KA, offA)
    callsB = pack_calls(KB, offB)
    # per tile: which call index covers it
    tcallA = np.zeros(NT, int)
    for i, (c0, nc_, t0, nt_) in enumerate(callsA):
        tcallA[t0:t0 + nt_] = i
    tcallB = np.zeros(NT, int)
    for i, (c0, nc_, t0, nt_) in enumerate(callsB):
        tcallB[t0:t0 + nt_] = i

    # pooling one-hot per core [128, NT, 64] and counts
    batch = np.asarray(batch)
    node = np.arange(NPAD)
    valid = node < N_NODES
    gid = np.where(valid, batch[np.minimum(node, N_NODES - 1)], 0)
    onehot = np.zeros((NPAD, N_GRAPHS), f32)
    onehot[valid, gid[valid]] = 1.0
    bp = onehot.reshape(N_CORES, NT, 128, N_GRAPHS).transpose(0, 2, 1, 3)
    bp = np.ascontiguousarray(bp).astype(bf16)
    cnt_g = np.bincount(batch, minlength=N_GRAPHS).astype(f32)
    cntinv = (1.0 / np.maximum(cnt_g, 1.0)).reshape(N_GRAPHS, 1).astype(f32)

    # weights
    W0f, W1f, W2f = np.asarray(W0, f32), np.asarray(W1, f32), np.asarray(W2, f32)
    Wexp0 = np.zeros((18, MID), f32)
    for h in range(HEADS):
        Wexp0[h * 6:(h + 1) * 6, h * OPH:(h + 1) * OPH] = \
            W0f[:, h * OPH:(h + 1) * OPH]
    rhs1 = np.concatenate([W1f, W1f @ amat(as1), W1f @ amat(ad1)], axis=1).astype(bf16)
    rhs2 = np.concatenate([W2f, W2f @ amat(as2), W2f @ amat(ad2)], axis=1).astype(bf16)

    bb0 = np.tile(np.asarray(b0, f32)[None, :], (128, 1)).astype(bf16)
    bb1 = np.tile(np.asarray(b1, f32)[None, :], (128, 1)).astype(bf16)
    bb2 = np.tile(np.asarray(b2, f32)[None, :], (128, 1)).astype(bf16)

    locT = np.asarray(loc, f32).reshape(N_GRAPHS * N_LOCS, 2).T.astype(bf16)
    ident = np.eye(128, dtype=bf16)

    common = dict(
        idx_meta=None,
        Wexp0=Wexp0.astype(bf16),
        rhs1a=rhs1[:128].copy(), rhs1b=rhs1[128:].copy(),
        rhs2a=rhs2[:128].copy(), rhs2b=rhs2[128:].copy(),
        bb0=bb0, bb1=bb1, bb2=bb2, cntinv=cntinv,
        locT=locT, wl1=np.asarray(Wl1, f32).astype(bf16),
        bl1c=np.asarray(bl1, f32).reshape(-1, 1),
        wl2=np.asarray(Wl2, f32).astype(bf16), bl2c=np.asarray(bl2, f32).reshape(-1, 1),
        ident=ident,
    )
    del common["idx_meta"]
    in_maps = []
    for c in range(N_CORES):
        m = dict(common)
        m["idxA"] = idxA_dev[c]
        m["idxB"] = idxB_dev[c]
        m["pdst"] = pdst_dev[c]
        m["pedge"] = pedge_dev[c]
        m["xe"] = xe_dev[c]
        m["x_own"] = x_own[c].astype(bf16)
        m["es0_own"] = es0_own[c].astype(f32)
        m["ed0_own"] = ed0_own[c].astype(bf16)
        m["bpool"] = bp[c]
        in_maps.append(m)

    meta = (KA, KB, tuple(offA.tolist()), tuple(offB.tolist()),
            tuple(offT.tolist()), tuple(callsA), tuple(callsB),
            tuple(tcallA.tolist()), tuple(tcallB.tolist()))
    return in_maps, meta


def _build(meta):
    (KA, KB, offA, offB, offT, callsA, callsB, tcallA, tcallB) = meta
    CA, CB = offA[-1], offB[-1]
    TOT = offT[-1]
    MAXK = max(KA[t] + KB[t] for t in range(NT))
    _install_tile_patch()
    import concourse.bass as bass
    import concourse.mybir as mybir
    import concourse.tile as tile
    from concourse import library_config
    from concourse.library_overlay import lower_extended_insts

    f32 = mybir.dt.float32
    bf16 = mybir.dt.bfloat16
    fp16 = mybir.dt.float16
    fp8 = mybir.dt.float8e4
    i16 = mybir.dt.int16
    i8 = mybir.dt.int8
    u8 = mybir.dt.uint8
    AF = mybir.ActivationFunctionType
    AO = mybir.AluOpType

    nc = bass.Bass(num_devices=N_CORES, dynamic_dma_scratch_size=DMA_SCRATCH)

    inp = {}
    for name, shape, dt in [
        ("idxA", [128, CA * 8], i16), ("idxB", [128, CB * 8], i16),
        ("pdst", [128, TOT * 128], fp8), ("pedge", [128, TOT * 128], fp8),
        ("xe", [128, TOT, 12], bf16),
        ("x_own", [128, NT, 6], bf16),
        ("es0_own", [128, NT, 3], f32), ("ed0_own", [128, NT, 3], bf16),
        ("bpool", [128, NT, N_GRAPHS], bf16), ("cntinv", [N_GRAPHS, 1], f32),
        ("Wexp0", [18, MID], bf16),
        ("rhs1a", [128, MID + 6], bf16), ("rhs1b", [1, MID + 6], bf16),
        ("rhs2a", [128, GH + 2], bf16), ("rhs2b", [1, GH + 2], bf16),
        ("bb0", [128, MID], bf16), ("bb1", [128, MID], bf16),
        ("bb2", [128, GH], bf16),
        ("locT", [2, N_GRAPHS * N_LOCS], bf16), ("wl1", [2, MLPH], bf16),
        ("bl1c", [MLPH, 1], f32), ("wl2", [MLPH, GH], bf16), ("bl2c", [GH, 1], f32),
        ("ident", [128, 128], bf16),
    ]:
        inp[name] = nc.dram_tensor(name, shape, dt, kind="ExternalInput")

    out = nc.dram_tensor("out", [N_GRAPHS, GH * 2], f32, kind="ExternalOutput")

    # half tables per layer (u8 rows of ROWB bytes)
    Ta = [nc.dram_tensor(f"T{l}a", [A_TOT, ROWB], u8, kind="Internal",
                         addr_space="Shared") for l in (1, 2)]
    Tb = [nc.dram_tensor(f"T{l}b", [B_TOT, ROWB], u8, kind="Internal",
                         addr_space="Shared") for l in (1, 2)]
    Tsha = [nc.dram_tensor(f"Tsh{l}a", [HA, ROWB], u8, kind="Internal")
            for l in (1, 2)]
    Tshb = [nc.dram_tensor(f"Tsh{l}b", [HB, ROWB], u8, kind="Internal")
            for l in (1, 2)]
    s_in = nc.dram_tensor("s_in", [N_GRAPHS, GH], f32, kind="Internal")
    s_out = nc.dram_tensor("s_out", [N_GRAPHS, GH], f32, kind="Internal",
                           addr_space="Shared")

    RG = [list(range(N_CORES))]

    with tile.TileContext(nc) as tc:
        with tc.tile_pool(name="const", bufs=1) as cp, \
             tc.tile_pool(name="own", bufs=1) as ownp, \
             tc.tile_pool(name="ga", bufs=2) as gap, \
             tc.tile_pool(name="gb", bufs=2) as gbp, \
             tc.tile_pool(name="pp", bufs=2) as ppool, \
             tc.tile_pool(name="sb", bufs=3) as sb, \
             tc.tile_pool(name="mlp", bufs=1) as mlppool, \
             tc.tile_pool(name="ps", bufs=2, space="PSUM") as ps, \
             tc.tile_pool(name="pse", bufs=2, space="PSUM") as pse, \
             tc.tile_pool(name="psacc", bufs=2, space="PSUM") as psacc, \
             tc.tile_pool(name="pspool", bufs=1, space="PSUM") as pspool:

            nc.gpsimd.load_library(library_config.mlp)

            # distinct gather sizes -> regs
            sizes = sorted({nc_ * 128 for _, nc_, _, _ in callsA} |
                           {nc_ * 128 for _, nc_, _, _ in callsB})
            nidx_regs = {n: nc.gpsimd.to_reg(n) for n in sizes}

            ident = cp.tile([128, 128], bf16)
            nc.sync.dma_start(ident[:], inp["ident"][:, :])

            wexp0 = cp.tile([18, MID], bf16, tag="wexp0")
            nc.sync.dma_start(wexp0[:], inp["Wexp0"][:, :])
            rhs_a, rhs_b = {}, {}
            for l, nma, nmb in [(1, "rhs1a", "rhs1b"), (2, "rhs2a", "rhs2b")]:
                wdt = inp[nma].shape[1]
                ta = cp.tile([128, wdt], bf16, tag=f"rhsa{l}")
                nc.sync.dma_start(ta[:], inp[nma][:, :])
                tb = cp.tile([1, wdt], bf16, tag=f"rhsb{l}")
                nc.sync.dma_start(tb[:], inp[nmb][:, :])
                rhs_a[l], rhs_b[l] = ta, tb
            bb_sb = []
            for l, nm in enumerate(["bb0", "bb1", "bb2"]):
                t = cp.tile(list(inp[nm].shape), bf16, tag=f"bb{l}")
                nc.sync.dma_start(t[:], inp[nm][:, :])
                bb_sb.append(t)
            bp_sb = cp.tile([128, NT, N_GRAPHS], bf16, tag="bpall")
            nc.sync.dma_start(bp_sb[:], inp["bpool"][:, :, :])

            xown = cp.tile([128, NT, 6], bf16, tag="xown")
            nc.sync.dma_start(xown[:], inp["x_own"][:, :, :])
            es0o = cp.tile([128, NT, 3], f32, tag="es0o")
            nc.sync.dma_start(es0o[:], inp["es0_own"][:, :, :])
            ed0o = cp.tile([128, NT, 3], bf16, tag="ed0o")
            nc.sync.dma_start(ed0o[:], inp["ed0_own"][:, :, :])

            # own-shard table rows for layers 1,2 (built in prev layer)
            own_t = [ownp.tile([128, NT, ROWB], u8, tag=f"own{l}",
                               name=f"own{l}") for l in (1, 2)]
            # es/ed caches for self terms + ed broadcast, layers 1,2
            esed = {}
            for l, hh in ((1, 3), (2, 1)):
                esed[l] = (ownp.tile([128, NT, hh], bf16, tag=f"esc{l}",
                                     name=f"esc{l}"),
                           ownp.tile([128, NT, hh], bf16, tag=f"edc{l}",
                                     name=f"edc{l}"))

            # ---------------- loc MLP (independent; issue early) ------------
            locT = mlppool.tile([2, N_GRAPHS * N_LOCS], bf16)
            nc.sync.dma_start(locT[:], inp["locT"][:, :])
            wl1 = mlppool.tile([2, MLPH], bf16)
            nc.sync.dma_start(wl1[:], inp["wl1"][:, :])
            bl1c = mlppool.tile([128, 2], f32)
            nc.sync.dma_start(bl1c[:], inp["bl1c"][:, 0:1].rearrange(
                "(h p) o -> p (h o)", p=128))
            wl2a = mlppool.tile([128, GH], bf16)
            nc.sync.dma_start(wl2a[:], inp["wl2"][0:128, :])
            wl2b = mlppool.tile([128, GH], bf16)
            nc.sync.dma_start(wl2b[:], inp["wl2"][128:256, :])
            bl2c = mlppool.tile([GH, 1], f32)
            nc.sync.dma_start(bl2c[:], inp["bl2c"][:, :])
            identf = mlppool.tile([128, 128], f32)
            nc.vector.tensor_copy(identf[:], ident[:])
            mid_sb = [mlppool.tile([128, N_GRAPHS * N_LOCS], bf16,
                                   tag=f"mid{h}", name=f"mid{h}")
                      for h in range(2)]
            CH = 400
            nch = (N_GRAPHS * N_LOCS) // CH
            for h in range(2):
                for c in range(nch):
                    pm = ps.tile([128, CH], f32, space="PSUM", tag="pscr")
                    nc.tensor.matmul(pm[:], lhsT=wl1[:, h * 128:(h + 1) * 128],
                                     rhs=locT[:, c * CH:(c + 1) * CH],
                                     start=True, stop=True)
                    nc.scalar.activation(mid_sb[h][:, c * CH:(c + 1) * CH], pm[:],
                                         AF.Tanh, bias=bl1c[:, h:h + 1])
            lpT = mlppool.tile([128, N_GRAPHS], f32)
            for c in range(nch):
                po = ps.tile([128, CH], f32, space="PSUM", tag="pscr")
                nc.tensor.matmul(po[:], lhsT=wl2a[:],
                                 rhs=mid_sb[0][:, c * CH:(c + 1) * CH],
                                 start=True, stop=False)
                nc.tensor.matmul(po[:], lhsT=wl2b[:],
                                 rhs=mid_sb[1][:, c * CH:(c + 1) * CH],
                                 start=False, stop=True)
                ng = CH // N_LOCS
                nc.vector.reduce_sum(
                    lpT[:, c * ng:(c + 1) * ng],
                    po[:].rearrange("p (g l) -> p g l", l=N_LOCS),
                    axis=mybir.AxisListType.X)
            lpT2 = mlppool.tile([128, N_GRAPHS], f32)
            nc.vector.tensor_scalar(lpT2[:], lpT[:], 1.0 / N_LOCS, bl2c[:],
                                    AO.mult, AO.add)
            plp = ps.tile([N_GRAPHS, 128], f32, space="PSUM", tag="pscr")
            nc.tensor.matmul(plp[:], lhsT=lpT2[:], rhs=identf[:],
                             start=True, stop=True)
            out_sb = mlppool.tile([N_GRAPHS, 2 * GH], f32)
            nc.vector.tensor_copy(out_sb[:, GH:], plp[:])

            psum_S = pspool.tile([N_GRAPHS, GH], f32, space="PSUM")

            # ---- shared epilogue helper: table build from hin [128, F] ----
            def build_table(l, t, hin, Fin):
                """hin [128, Fin] bf16 -> quantized row of T_{l} for tile t."""
                ph1 = ps.tile([128, 128], f32, space="PSUM", tag="pscr")
                nc.tensor.matmul(ph1[:], lhsT=hin[:, 0:128], rhs=ident[:],
                                 start=True, stop=True)
                hTa = sb.tile([128, 128], bf16, tag="hTa")
                nc.vector.tensor_copy(hTa[:], ph1[:])
                ph2 = ps.tile([1, 128], f32, space="PSUM", tag="pscr")
                nc.tensor.matmul(ph2[:], lhsT=hin[:, 128:Fin], rhs=ident[:],
                                 start=True, stop=True)
                hTb = sb.tile([1, 128], bf16, tag="hTb")
                nc.vector.tensor_copy(hTb[:], ph2[:])
                F2 = MID if l == 1 else GH      # next table's feature width
                H2 = 3 if l == 1 else 1
                W2_ = F2 + 2 * H2
                ptab = ps.tile([128, W2_], f32, space="PSUM", tag="pscr")
                nc.tensor.matmul(ptab[:], lhsT=hTa[:], rhs=rhs_a[l][:, :W2_],
                                 start=True, stop=False)
                nc.tensor.matmul(ptab[:], lhsT=hTb[:], rhs=rhs_b[l][:, :W2_],
                                 start=False, stop=True)
                # quantize h' -> int8 with fp16 scale; cache es/ed
                am = sb.tile([128, 1], f32, tag="am")
                nc.vector.reduce_max(am[:], ptab[:, 0:F2],
                                     axis=mybir.AxisListType.X,
                                     apply_absolute_value=True)
                sc = sb.tile([128, 1], f32, tag="sc")
                nc.vector.tensor_scalar(sc[:], am[:], 1.0 / 127.0, 1e-20,
                                        AO.mult, AO.max)
                sci = sb.tile([128, 1], f32, tag="sci")
                nc.vector.reciprocal(sci[:], sc[:])
                stg = own_t[l - 1][:, t, :]          # [128, ROWB] u8 slice
                nc.vector.memset(stg[:, 137:ROWB], 0.0)
                nc.vector.memset(stg[:, 4:8], 0.0)
                nc.vector.tensor_copy(stg[:, 0:2].bitcast(fp16), sc[:])
                nc.vector.tensor_copy(stg[:, 2:2 + 2 * H2].bitcast(fp16),
                                      ptab[:, F2:F2 + H2])
                nc.scalar.activation(stg[:, 8:8 + F2].bitcast(i8),
                                     ptab[:, 0:F2], AF.Copy, scale=sci[:])
                if F2 < MID:
                    nc.vector.memset(stg[:, 8 + F2:137], 0.0)
                esc, edc = esed[l]
                nc.vector.tensor_copy(esc[:, t, :], ptab[:, F2:F2 + H2])
                nc.vector.tensor_copy(edc[:, t, :], ptab[:, F2 + H2:F2 + 2 * H2])
                # ship to shard table region
                if t < NT_A:
                    nc.sync.dma_start(Tsha[l - 1][t * 128:(t + 1) * 128, :], stg[:])
                else:
                    tt = t - NT_A
                    nc.sync.dma_start(Tshb[l - 1][tt * 128:(tt + 1) * 128, :], stg[:])

            # ================= layer 0 (no gather) =================
            for t in range(NT):
                KAt, KBt, Kt = KA[t], KB[t], KA[t] + KB[t]
                f0 = offT[t]
                pd = ppool.tile([128, MAXK * 128], fp8, tag="pd")
                nc.sync.dma_start(pd[:, 0:Kt * 128],
                                  inp["pdst"][:, f0 * 128:(f0 + Kt) * 128])
                pe_ = ppool.tile([128, MAXK * 128], fp8, tag="pe")
                nc.scalar.dma_start(pe_[:, 0:Kt * 128],
                                    inp["pedge"][:, f0 * 128:(f0 + Kt) * 128])
                xet = sb.tile([128, MAXK, 12], bf16, tag="xet")
                nc.sync.dma_start(xet[:, 0:Kt, :], inp["xe"][:, f0:f0 + Kt, :])
                # ed broadcast
                pede = pse.tile([128, MAXK, 3], f32, space="PSUM",
                                tag="pede")
                for j in range(Kt):
                    nc.tensor.matmul(pede[:, j, :],
                                     lhsT=pd[:, j * 128:(j + 1) * 128],
                                     rhs=ed0o[:, t, :], start=True, stop=True)
                esum = sb.tile([128, MAXK, 3], f32, tag="esum")
                nc.vector.tensor_tensor(out=esum[:, 0:Kt, :],
                                        in0=xet[:, 0:Kt, 6:9],
                                        in1=pede[:, 0:Kt, :], op=AO.add)
                lk = sb.tile([128, MAXK, 3], f32, tag="lk")
                nc.scalar.activation(lk[:, 0:Kt, :], esum[:, 0:Kt, :],
                                     AF.Prelu, alpha=NEG)
                w = sb.tile([128, MAXK, 3], bf16, tag="w")
                nc.scalar.activation(w[:, 0:Kt, :], lk[:, 0:Kt, :], AF.Exp)
                rt = sb.tile([128, MAXK, 21], bf16, tag="rt")
                for h in range(3):
                    nc.vector.tensor_tensor(
                        out=rt[:, 0:Kt, h * 6:(h + 1) * 6],
                        in0=xet[:, 0:Kt, 0:6],
                        in1=w[:, 0:Kt, h:h + 1].to_broadcast([128, Kt, 6]),
                        op=AO.mult)
                nc.vector.tensor_copy(rt[:, 0:Kt, 18:21], w[:, 0:Kt, :])
                acc = psacc.tile([128, 21], f32, space="PSUM", tag="acc")
                for j in range(Kt):
                    nc.tensor.matmul(acc[:], lhsT=pe_[:, j * 128:(j + 1) * 128],
                                     rhs=rt[:, j, :],
                                     start=(j == 0), stop=(j == Kt - 1))
                # self-loop term + normalize
                se = sb.tile([128, 3], f32, tag="se")
                nc.vector.tensor_tensor(out=se[:], in0=es0o[:, t, :],
                                        in1=ed0o[:, t, :], op=AO.add)
                slk = sb.tile([128, 3], f32, tag="slk")
                nc.scalar.activation(slk[:], se[:], AF.Prelu, alpha=NEG)
                wself = sb.tile([128, 3], f32, tag="wself")
                nc.scalar.activation(wself[:], slk[:], AF.Exp)
                selfm = sb.tile([128, 21], f32, tag="selfm")
                for h in range(3):
                    nc.vector.tensor_scalar_mul(
                        selfm[:, h * 6:(h + 1) * 6], xown[:, t, :],
                        wself[:, h:h + 1])
                nc.vector.tensor_copy(selfm[:, 18:21], wself[:])
                accs = sb.tile([128, 21], f32, tag="accs")
                nc.vector.tensor_tensor(out=accs[:], in0=acc[:], in1=selfm[:],
                                        op=AO.add)
                zr = sb.tile([128, 3], f32, tag="zr")
                nc.vector.reciprocal(zr[:], accs[:, 18:21])
                un = sb.tile([128, 18], bf16, tag="un")
                for h in range(3):
                    nc.scalar.activation(un[:, h * 6:(h + 1) * 6],
                                         accs[:, h * 6:(h + 1) * 6],
                                         AF.Copy, scale=zr[:, h:h + 1])
                pt_ = ps.tile([18, 128], f32, space="PSUM", tag="pscr")
                nc.tensor.matmul(pt_[:], lhsT=un[:], rhs=ident[:],
                                 start=True, stop=True)
                accT = sb.tile([18, 128], bf16, tag="accT")
                nc.vector.tensor_copy(accT[:], pt_[:])
                h0ps = ps.tile([128, MID], f32, space="PSUM", tag="pscr")
                nc.tensor.matmul(h0ps[:], lhsT=accT[:], rhs=wexp0[:],
                                 start=True, stop=True)
                hin = sb.tile([128, MID], bf16, tag="hin")
                nc.vector.tensor_add(hin[:], h0ps[:], bb_sb[0][:, :MID])
                nc.scalar.activation(hin[:], hin[:], AF.Prelu, alpha=NEG)
                build_table(1, t, hin, MID)
                if t == NT_A - 1:
                    nc.gpsimd.collective_compute(
                        "AllGather", AO.bypass, replica_groups=RG,
                        ins=[Tsha[0][:, :]], outs=[Ta[0][:, :]])
            nc.gpsimd.collective_compute(
                "AllGather", AO.bypass, replica_groups=RG,
                ins=[Tshb[0][:, :]], outs=[Tb[0][:, :]])

            # ================= layers 1, 2 (gathered) =================
            for l in (1, 2):
                F = MID if l == 1 else GH
                Hh = 3 if l == 1 else 1
                O = F // Hh
                RW = F + Hh
                tabA = Ta[l - 1]
                tabB = Tb[l - 1]
                esc, edc = esed[l]
                ownl = own_t[l - 1]

                gbufA = [gap.tile([128, CALL_COLS, ROWB], u8,
                                  tag=f"gA{i}", name=f"gA{i}_{l}")
                         for i in range(2)]
                gbufB = [gbp.tile([128, CALL_COLS, ROWB], u8,
                                  tag=f"gB{i}", name=f"gB{i}_{l}")
                         for i in range(2)]
                issuedA = [False] * len(callsA)
                issuedB = [False] * len(callsB)

                def issue(calls, issued, idx_inp, tab, gbufs, i):
                    if issued[i]:
                        return
                    issued[i] = True
                    c0, ncol, _, _ = calls[i]
                    n = ncol * 128
                    # load idx slice (transient)
                    it = sb.tile([128, CALL_COLS * 8], i16, tag="idxt")
                    nc.sync.dma_start(it[:, 0:ncol * 8],
                                      idx_inp[:, c0 * 8:(c0 + ncol) * 8])
                    # single_packet=False: the SDMA packet ceiling is <=64
                    # descriptors; a 36-col call emits ~290 descs per engine.
                    nc.gpsimd.dma_gather(
                        out_ap=gbufs[i % 2][:, 0:ncol, :], in_ap=tab[:, :],
                        idxs_ap=it[:, 0:ncol * 8], num_idxs=n,
                        num_idxs_reg=nidx_regs[n], elem_size=ROWB,
                        single_packet=False)

                for t in range(NT):
                    KAt, KBt, Kt = KA[t], KB[t], KA[t] + KB[t]
                    ia, ib = tcallA[t], tcallB[t]
                    issue(callsA, issuedA, inp["idxA"], tabA, gbufA, ia)
                    issue(callsB, issuedB, inp["idxB"], tabB, gbufB, ib)
                    # also prefetch next calls
                    if ia + 1 < len(callsA):
                        issue(callsA, issuedA, inp["idxA"], tabA, gbufA, ia + 1)
                    if ib + 1 < len(callsB):
                        issue(callsB, issuedB, inp["idxB"], tabB, gbufB, ib + 1)
                    gA = gbufA[ia % 2][:, offA[t] - callsA[ia][0]:
                                       offA[t] - callsA[ia][0] + KAt, :]
                    gB = gbufB[ib % 2][:, offB[t] - callsB[ib][0]:
                                       offB[t] - callsB[ib][0] + KBt, :]

                    f0 = offT[t]
                    pd = ppool.tile([128, MAXK * 128], fp8, tag="pd")
                    nc.sync.dma_start(pd[:, 0:Kt * 128],
                                      inp["pdst"][:, f0 * 128:(f0 + Kt) * 128])
                    pe_ = ppool.tile([128, MAXK * 128], fp8, tag="pe")
                    nc.scalar.dma_start(pe_[:, 0:Kt * 128],
                                        inp["pedge"][:, f0 * 128:(f0 + Kt) * 128])
                    pede = pse.tile([128, MAXK, 3], f32, space="PSUM",
                                    tag="pede")
                    for j in range(Kt):
                        nc.tensor.matmul(pede[:, j, :Hh],
                                         lhsT=pd[:, j * 128:(j + 1) * 128],
                                         rhs=edc[:, t, :], start=True, stop=True)
                    # w per piece (A then B)
                    esum = sb.tile([128, MAXK, 3], f32, tag="esum")
                    wt = sb.tile([128, MAXK, 3], bf16, tag="w")
                    wsc = sb.tile([128, MAXK, 3], bf16, tag="wsc")
                    rt = sb.tile([128, MAXK, MID + 3], bf16, tag="rt")
                    for g, ko, kn in ((gA, 0, KAt), (gB, KAt, KBt)):
                        nc.vector.tensor_tensor(
                            out=esum[:, ko:ko + kn, :Hh],
                            in0=g[:, :, 2:2 + 2 * Hh].bitcast(fp16),
                            in1=pede[:, ko:ko + kn, :Hh], op=AO.add)
                        nc.scalar.activation(esum[:, ko:ko + kn, :Hh],
                                             esum[:, ko:ko + kn, :Hh],
                                             AF.Prelu, alpha=NEG)
                        nc.scalar.activation(wt[:, ko:ko + kn, :Hh],
                                             esum[:, ko:ko + kn, :Hh], AF.Exp)
                        nc.vector.tensor_tensor(
                            out=wsc[:, ko:ko + kn, :Hh],
                            in0=wt[:, ko:ko + kn, :Hh],
                            in1=g[:, :, 0:2].bitcast(fp16).to_broadcast(
                                [128, kn, Hh]),
                            op=AO.mult)
                        for h in range(Hh):
                            nc.vector.tensor_tensor(
                                out=rt[:, ko:ko + kn, h * O:(h + 1) * O],
                                in0=g[:, :, 8 + h * O:8 + (h + 1) * O].bitcast(i8),
                                in1=wsc[:, ko:ko + kn, h:h + 1].to_broadcast(
                                    [128, kn, O]),
                                op=AO.mult)
                        nc.vector.tensor_copy(rt[:, ko:ko + kn, F:F + Hh],
                                              wt[:, ko:ko + kn, :Hh])
                    acc = psacc.tile([128, MID + 3], f32, space="PSUM", tag="acc")
                    for j in range(Kt):
                        nc.tensor.matmul(acc[:, :RW],
                                         lhsT=pe_[:, j * 128:(j + 1) * 128],
                                         rhs=rt[:, j, :RW],
                                         start=(j == 0), stop=(j == Kt - 1))
                    # self-loop + normalize
                    se = sb.tile([128, 3], f32, tag="se")
                    nc.vector.tensor_tensor(out=se[:, :Hh], in0=esc[:, t, :],
                                            in1=edc[:, t, :], op=AO.add)
                    nc.scalar.activation(se[:, :Hh], se[:, :Hh],
                                         AF.Prelu, alpha=NEG)
                    wself = sb.tile([128, 3], f32, tag="wself")
                    nc.scalar.activation(wself[:, :Hh], se[:, :Hh], AF.Exp)
                    wssc = sb.tile([128, 3], f32, tag="wssc")
                    nc.vector.tensor_tensor(
                        out=wssc[:, :Hh], in0=wself[:, :Hh],
                        in1=ownl[:, t, 0:2].bitcast(fp16).to_broadcast([128, Hh]),
                        op=AO.mult)
                    selfm = sb.tile([128, MID + 3], f32, tag="selfm")
                    for h in range(Hh):
                        nc.vector.tensor_scalar_mul(
                            selfm[:, h * O:(h + 1) * O],
                            ownl[:, t, 8 + h * O:8 + (h + 1) * O].bitcast(i8),
                            wssc[:, h:h + 1])
                    nc.vector.tensor_copy(selfm[:, F:F + Hh], wself[:, :Hh])
                    accs = sb.tile([128, MID + 3], f32, tag="accs")
                    nc.vector.tensor_tensor(out=accs[:, :RW], in0=acc[:, :RW],
                                            in1=selfm[:, :RW], op=AO.add)
                    zr = sb.tile([128, 3], f32, tag="zr")
                    nc.vector.reciprocal(zr[:, :Hh], accs[:, F:F + Hh])
                    ob = sb.tile([128, MID], bf16, tag="ob")
                    for h in range(Hh):
                        nc.scalar.activation(ob[:, h * O:(h + 1) * O],
                                             accs[:, h * O:(h + 1) * O],
                                             AF.Copy, scale=zr[:, h:h + 1])
                    if l == 1:
                        hin = sb.tile([128, MID], bf16, tag="hin")
                        nc.vector.tensor_add(hin[:], ob[:], bb_sb[1][:, :MID])
                        nc.scalar.activation(hin[:], hin[:], AF.Prelu, alpha=NEG)
                        build_table(2, t, hin, MID)
                        if t == NT_A - 1:
                            nc.gpsimd.collective_compute(
                                "AllGather", AO.bypass, replica_groups=RG,
                                ins=[Tsha[1][:, :]], outs=[Ta[1][:, :]])
                    else:
                        ob2 = sb.tile([128, GH], bf16, tag="ob2")
                        nc.vector.tensor_add(ob2[:], ob[:, :GH], bb_sb[2][:, :GH])
                        nc.tensor.matmul(psum_S[:], lhsT=bp_sb[:, t, :],
                                         rhs=ob2[:],
                                         start=(t == 0), stop=(t == NT - 1))
                if l == 1:
                    nc.gpsimd.collective_compute(
                        "AllGather", AO.bypass, replica_groups=RG,
                        ins=[Tshb[1][:, :]], outs=[Tb[1][:, :]])

            # pooling: AllReduce of per-shard sums, then divide by counts
            ssb = sb.tile([N_GRAPHS, GH], f32, tag="ssb")
            nc.vector.tensor_copy(ssb[:], psum_S[:])
            nc.sync.dma_start(s_in[:, :], ssb[:])
            nc.gpsimd.collective_compute(
                "AllReduce", AO.add, replica_groups=RG,
                ins=[s_in[:, :]], outs=[s_out[:, :]])
            sfull = sb.tile([N_GRAPHS, GH], f32, tag="sfull")
            nc.sync.dma_start(sfull[:], s_out[:, :])
            civ = sb.tile([N_GRAPHS, 1], f32, tag="civ")
            nc.sync.dma_start(civ[:], inp["cntinv"][:, :])
            nc.vector.tensor_scalar_mul(out_sb[:, 0:GH], sfull[:], civ[:])
            nc.sync.dma_start(out[:, :], out_sb[:])

    lower_extended_insts(nc)
    return nc


def kernel(**inputs):
    key = "k"
    in_maps, meta = _host_prep(**inputs)
    if key not in _CACHE or _CACHE[key][1] != meta:
        nc = _build(meta)
        _CACHE[key] = (nc, meta)
    nc = _CACHE[key][0]
    from concourse.bass_utils import run_bass_kernel_spmd
    res = run_bass_kernel_spmd(nc, in_maps, core_ids=list(range(N_CORES)))
    return np.asarray(res.results[0]["out"])


# revision 9
# speedup vs baseline: 1.5859x; 1.0898x over previous
"""GAT backbone (3-layer GATConv + graph pooling + loc-MLP) on 8 Trainium2
NeuronCores.

Strategy (v2): dst-sharded edges, gather-minimized.
- Layer 0 messages are rank-6 in x: the host pre-expands [x[src]|es0] per
  edge slot (sequential DMA, no gather), the scatter runs at width 21
  ([w_h*x | w]), and a per-tile Wexp matmul recovers the 129-wide GAT0
  output. No T0 table, no layer-0 dma_gather.
- Layers 1/2 gather 256-byte table rows [fp16 scale | fp16 es | int8 h']
  via dma_gather in large tile-packed calls (~4.6K indices each, enlarged
  SWDGE descriptor ring) from two half-tables T_a/T_b. Node positions are
  permuted so half A = all cores' first 24 dst-tiles: each half's
  AllGather unblocks its gather calls independently.
- Self-loops are excluded from edge lists and added analytically in each
  tile epilogue (own-shard rows + es/ed caches stay in SBUF).
- Per-edge attention: one-hot fp8 matmuls broadcast ed to edges (pdst) and
  scatter [w*h'|w] into PSUM (pedge), as in v1.
"""
import numpy as np

# ---------------------------------------------------------------------------
# runtime patch: this walrus build accepts at most ONE sync-wait command per
# instruction; Tile attaches several. Split extras into single-wait NOPs.
# ---------------------------------------------------------------------------
_PATCHED = [False]


def _install_tile_patch():
    if _PATCHED[0]:
        return
    import concourse.mybir as mybir
    from concourse.tile import TileContext
    from concourse.vector_clock import ScopedClock

    ctr = [0]

    def _split(insts):
        new = []
        for inst in insts:
            si = getattr(inst, "sync_info", None)
            try:
                ow = si.on_wait if si is not None else None
            except Exception:
                ow = None
            if ow is not None and len(ow) > 1:
                waits = list(ow)
                for w in waits[:-1]:
                    ctr[0] += 1
                    nop = mybir.InstNoOp(name=f"wsplit-{ctr[0]}", ins=[], outs=[])
                    nop.engine = inst.engine
                    nop.sync_info = mybir.SyncInfo(on_wait=[w], on_update=[])
                    new.append(nop)
                si.on_wait = waits[-1:]
            new.append(inst)
        insts[:] = new

    orig_lower = TileContext._lower_ordered_insts

    def patched_lower(self, ordered):
        for insts in ordered.values():
            _split(insts)
        return orig_lower(self, ordered)

    def patched_drain(self, tick_clock, wait_clock):
        drain_inst = self.nc.sync.drain()
        wait_clock.add_sem_waits(
            drain_inst.ins, ScopedClock({None: tick_clock.global_clock})
        )
        si = drain_inst.ins.sync_info
        if si is not None and si.on_wait and len(si.on_wait) > 1:
            waits = list(si.on_wait)
            si.on_wait = waits[:1]
            for w in waits[1:]:
                extra = self.nc.sync.drain()
                esi = extra.ins.sync_info
                if esi is None:
                    extra.ins.sync_info = mybir.SyncInfo(on_wait=[w], on_update=[])
                else:
                    esi.on_wait = [w]
        self.nc.all_engine_barrier()
        assert self.sems is not None
        popped = self.nc._tile_sem_poison_stack.pop()
        assert popped is self._sem_poison
        self.nc.clear_and_free_semaphores(list(self.sems.allocated().values()))
        self.nc.all_engine_barrier()

    TileContext._lower_ordered_insts = patched_lower
    TileContext._drain_and_barrier = patched_drain
    _PATCHED[0] = True


# ---------------------------------------------------------------------------
# problem constants (hardcoded per contract)
# ---------------------------------------------------------------------------
N_NODES = 50000
N_EDGES = 800000
N_GRAPHS = 64
N_LOCS = 50
HEADS = 3
OPH = 43                    # out per head
MID = HEADS * OPH           # 129
GH = 128                    # gat hidden (layer 2 out)
MLPH = 256
NEG = 0.2
N_CORES = 8
SHARD = 6272                # 49 * 128 dst nodes per core
NT = SHARD // 128           # 49 node-tiles
NPAD = SHARD * N_CORES      # 50176
NT_A = 24                   # tiles per core in half A
NT_B = NT - NT_A            # 25
HA = NT_A * 128             # 3072 rows/core in half A
HB = NT_B * 128             # 3200
A_TOT = HA * N_CORES        # 24576 rows in table A
B_TOT = HB * N_CORES        # 25600 rows in table B
ROWB = 256                  # table row bytes
CALL_COLS = 36              # max 128-edge columns per dma_gather call
DMA_SCRATCH = 40960         # SWDGE descriptor ring bytes/partition

_CACHE = {}


def _pi(n):
    """table position of node n: half-A shards first, then half-B shards."""
    c, r = n // SHARD, n % SHARD
    return np.where(r < HA, c * HA + r, A_TOT + c * HB + (r - HA))


def _host_prep(x, loc, edge_index, batch, W0, as0, ad0, b0, W1, as1, ad1, b1,
               W2, as2, ad2, b2, Wl1, bl1, Wl2, bl2):
    import ml_dtypes
    f32 = np.float32
    bf16 = ml_dtypes.bfloat16
    fp8 = ml_dtypes.float8_e4m3

    def amat(a):
        h, o = a.shape
        m = np.zeros((h * o, h), f32)
        for i in range(h):
            m[i * o:(i + 1) * o, i] = a[i]
        return m

    x = np.asarray(x, f32)
    src = np.asarray(edge_index[0])
    dst = np.asarray(edge_index[1])
    p_src = _pi(src)

    core = dst // SHARD
    tile = (dst % SHARD) // 128
    isB = (p_src >= A_TOT).astype(np.int64)
    key = (core * NT + tile) * 2 + isB
    order = np.argsort(key, kind="stable")
    src_s, dst_s, psrc_s = src[order], dst[order], p_src[order]
    key_s = key[order]
    cnt = np.bincount(key_s, minlength=N_CORES * NT * 2)
    bounds = np.zeros(N_CORES * NT * 2 + 1, np.int64)
    np.cumsum(cnt, out=bounds[1:])
    nA = cnt[0::2].reshape(N_CORES, NT)
    nB = cnt[1::2].reshape(N_CORES, NT)
    KA = tuple(int(np.ceil(nA[:, t].max() / 128)) for t in range(NT))
    KB = tuple(int(np.ceil(nB[:, t].max() / 128)) for t in range(NT))
    offA = np.concatenate([[0], np.cumsum(KA)]).astype(int)   # [NT+1]
    offB = np.concatenate([[0], np.cumsum(KB)]).astype(int)
    CA, CB = int(offA[-1]), int(offB[-1])
    TOT = sum(KA[t] + KB[t] for t in range(NT))
    offT = np.concatenate([[0], np.cumsum([KA[t] + KB[t] for t in range(NT)])]).astype(int)

    # per-edge slot assignment -> idx (A/B spaces), rel-dst one-hots, xe
    idxA = np.zeros((N_CORES, CA * 128), np.int16)
    idxB = np.zeros((N_CORES, CB * 128), np.int16)
    rel = np.zeros((N_CORES, TOT * 128), np.int64)
    val = np.zeros((N_CORES, TOT * 128), bool)
    xe = np.zeros((N_CORES, TOT * 128, 12), f32)

    es0w = W0 @ amat(as0)                       # [6,3]
    ed0w = W0 @ amat(ad0)
    es0_node = x @ es0w                         # [N,3]
    ed0_node = x @ ed0w

    for c in range(N_CORES):
        for t in range(NT):
            g = (c * NT + t) * 2
            sA, eA = bounds[g], bounds[g + 1]
            sB, eB = bounds[g + 1], bounds[g + 2]
            na, nb = eA - sA, eB - sB
            # A slots
            a0 = offA[t] * 128
            idxA[c, a0:a0 + na] = psrc_s[sA:eA].astype(np.int16)
            fa = offT[t] * 128
            rel[c, fa:fa + na] = dst_s[sA:eA] % 128
            val[c, fa:fa + na] = True
            xe[c, fa:fa + na, 0:6] = x[src_s[sA:eA]]
            xe[c, fa:fa + na, 6:9] = es0_node[src_s[sA:eA]]
            # B slots
            b0_ = offB[t] * 128
            idxB[c, b0_:b0_ + nb] = (psrc_s[sB:eB] - A_TOT).astype(np.int16)
            fb = (offT[t] + KA[t]) * 128
            rel[c, fb:fb + nb] = dst_s[sB:eB] % 128
            val[c, fb:fb + nb] = True
            xe[c, fb:fb + nb, 0:6] = x[src_s[sB:eB]]
            xe[c, fb:fb + nb, 6:9] = es0_node[src_s[sB:eB]]

    # idx wrap: [C, 128, CA*8] (16-partition wrap replicated x8)
    def wrap_idx(a, C):
        iw = a.reshape(N_CORES, C * 8, 16).transpose(0, 2, 1)
        return np.ascontiguousarray(np.tile(iw, (1, 8, 1)))
    idxA_dev = wrap_idx(idxA, CA)
    idxB_dev = wrap_idx(idxB, CB)

    # one-hots: pdst [d, e], pedge [e, d] per 128-edge column
    rel_i = rel.reshape(N_CORES, TOT, 128)
    val_i = val.reshape(N_CORES, TOT, 128)
    eye8 = np.eye(128, dtype=fp8)
    oh = eye8[rel_i]                            # [C, TOT, 128e, 128d]
    oh[~val_i] = 0
    pdst_dev = np.ascontiguousarray(
        oh.transpose(0, 3, 1, 2)).reshape(N_CORES, 128, TOT * 128)
    pedge_dev = np.ascontiguousarray(
        oh.transpose(0, 2, 1, 3)).reshape(N_CORES, 128, TOT * 128)

    xe_dev = np.ascontiguousarray(
        xe.reshape(N_CORES, TOT, 128, 12).transpose(0, 2, 1, 3)).astype(bf16)

    # own-shard per-node data for self-loop terms (layer 0); pad nodes -> 0
    xpad = np.zeros((NPAD, 6), f32)
    xpad[:N_NODES] = x
    es0pad = np.zeros((NPAD, 3), f32)
    es0pad[:N_NODES] = es0_node
    ed0pad = np.zeros((NPAD, 3), f32)
    ed0pad[:N_NODES] = ed0_node
    own = np.arange(SHARD)
    x_own = np.zeros((N_CORES, 128, NT, 6), f32)
    es0_own = np.zeros((N_CORES, 128, NT, 3), f32)
    ed0_own = np.zeros((N_CORES, 128, NT, 3), f32)
    for c in range(N_CORES):
        nodes = c * SHARD + own                  # global dst ids
        x_own[c] = xpad[nodes].reshape(NT, 128, 6).transpose(1, 0, 2)
        es0_own[c] = es0pad[nodes].reshape(NT, 128, 3).transpose(1, 0, 2)
        ed0_own[c] = ed0pad[nodes].reshape(NT, 128, 3).transpose(1, 0, 2)

    # gather call packing: greedy tiles while sum K <= CALL_COLS
    def pack_calls(Ks, offs):
        calls = []   # (colstart, ncols, first_tile, ntiles)
        t = 0
        while t < NT:
            t0, c0 = t, offs[t]
            while t < NT and offs[t + 1] - c0 <= CALL_COLS:
                t += 1
            calls.append((int(c0), int(offs[t] - c0), t0, t - t0))
        return calls
    callsA = pack_calls(KA, offA)
    callsB = pack_calls(KB, offB)
    # per tile: which call index covers it
    tcallA = np.zeros(NT, int)
    for i, (c0, nc_, t0, nt_) in enumerate(callsA):
        tcallA[t0:t0 + nt_] = i
    tcallB = np.zeros(NT, int)
    for i, (c0, nc_, t0, nt_) in enumerate(callsB):
        tcallB[t0:t0 + nt_] = i

    # pooling one-hot per core [128, NT, 64] and counts
    batch = np.asarray(batch)
    node = np.arange(NPAD)
    valid = node < N_NODES
    gid = np.where(valid, batch[np.minimum(node, N_NODES - 1)], 0)
    onehot = np.zeros((NPAD, N_GRAPHS), f32)
    onehot[valid, gid[valid]] = 1.0
    bp = onehot.reshape(N_CORES, NT, 128, N_GRAPHS).transpose(0, 2, 1, 3)
    bp = np.ascontiguousarray(bp).astype(bf16)
    cnt_g = np.bincount(batch, minlength=N_GRAPHS).astype(f32)
    cntinv = (1.0 / np.maximum(cnt_g, 1.0)).reshape(N_GRAPHS, 1).astype(f32)

    # weights
    W0f, W1f, W2f = np.asarray(W0, f32), np.asarray(W1, f32), np.asarray(W2, f32)
    Wexp0 = np.zeros((18, MID), f32)
    for h in range(HEADS):
        Wexp0[h * 6:(h + 1) * 6, h * OPH:(h + 1) * OPH] = \
            W0f[:, h * OPH:(h + 1) * OPH]
    rhs1 = np.concatenate([W1f, W1f @ amat(as1), W1f @ amat(ad1)], axis=1).astype(bf16)
    rhs2 = np.concatenate([W2f, W2f @ amat(as2), W2f @ amat(ad2)], axis=1).astype(bf16)

    bb0 = np.tile(np.asarray(b0, f32)[None, :], (128, 1)).astype(bf16)
    bb1 = np.tile(np.asarray(b1, f32)[None, :], (128, 1)).astype(bf16)
    bb2 = np.tile(np.asarray(b2, f32)[None, :], (128, 1)).astype(bf16)

    locT = np.asarray(loc, f32).reshape(N_GRAPHS * N_LOCS, 2).T.astype(bf16)
    ident = np.eye(128, dtype=bf16)

    common = dict(
        idx_meta=None,
        Wexp0=Wexp0.astype(bf16),
        rhs1a=rhs1[:128].copy(), rhs1b=rhs1[128:].copy(),
        rhs2a=rhs2[:128].copy(), rhs2b=rhs2[128:].copy(),
        bb0=bb0, bb1=bb1, bb2=bb2, cntinv=cntinv,
        locT=locT, wl1=np.asarray(Wl1, f32).astype(bf16),
        bl1c=np.asarray(bl1, f32).reshape(-1, 1),
        wl2=np.asarray(Wl2, f32).astype(bf16), bl2c=np.asarray(bl2, f32).reshape(-1, 1),
        ident=ident,
    )
    del common["idx_meta"]
    in_maps = []
    for c in range(N_CORES):
        m = dict(common)
        m["idxA"] = idxA_dev[c]
        m["idxB"] = idxB_dev[c]
        m["pdst"] = pdst_dev[c]
        m["pedge"] = pedge_dev[c]
        m["xe"] = xe_dev[c]
        m["x_own"] = x_own[c].astype(bf16)
        m["es0_own"] = es0_own[c].astype(f32)
        m["ed0_own"] = ed0_own[c].astype(bf16)
        m["bpool"] = bp[c]
        in_maps.append(m)

    meta = (KA, KB, tuple(offA.tolist()), tuple(offB.tolist()),
            tuple(offT.tolist()), tuple(callsA), tuple(callsB),
            tuple(tcallA.tolist()), tuple(tcallB.tolist()))
    return in_maps, meta


def _build(meta):
    (KA, KB, offA, offB, offT, callsA, callsB, tcallA, tcallB) = meta
    CA, CB = offA[-1], offB[-1]
    TOT = offT[-1]
    MAXK = max(KA[t] + KB[t] for t in range(NT))
    _install_tile_patch()
    import concourse.bass as bass
    import concourse.mybir as mybir
    import concourse.tile as tile
    from concourse import library_config
    from concourse.library_overlay import lower_extended_insts

    f32 = mybir.dt.float32
    bf16 = mybir.dt.bfloat16
    fp16 = mybir.dt.float16
    fp8 = mybir.dt.float8e4
    i16 = mybir.dt.int16
    i8 = mybir.dt.int8
    u8 = mybir.dt.uint8
    AF = mybir.ActivationFunctionType
    AO = mybir.AluOpType

    nc = bass.Bass(num_devices=N_CORES, dynamic_dma_scratch_size=DMA_SCRATCH)

    inp = {}
    for name, shape, dt in [
        ("idxA", [128, CA * 8], i16), ("idxB", [128, CB * 8], i16),
        ("pdst", [128, TOT * 128], fp8), ("pedge", [128, TOT * 128], fp8),
        ("xe", [128, TOT, 12], bf16),
        ("x_own", [128, NT, 6], bf16),
        ("es0_own", [128, NT, 3], f32), ("ed0_own", [128, NT, 3], bf16),
        ("bpool", [128, NT, N_GRAPHS], bf16), ("cntinv", [N_GRAPHS, 1], f32),
        ("Wexp0", [18, MID], bf16),
        ("rhs1a", [128, MID + 6], bf16), ("rhs1b", [1, MID + 6], bf16),
        ("rhs2a", [128, GH + 2], bf16), ("rhs2b", [1, GH + 2], bf16),
        ("bb0", [128, MID], bf16), ("bb1", [128, MID], bf16),
        ("bb2", [128, GH], bf16),
        ("locT", [2, N_GRAPHS * N_LOCS], bf16), ("wl1", [2, MLPH], bf16),
        ("bl1c", [MLPH, 1], f32), ("wl2", [MLPH, GH], bf16), ("bl2c", [GH, 1], f32),
        ("ident", [128, 128], bf16),
    ]:
        inp[name] = nc.dram_tensor(name, shape, dt, kind="ExternalInput")

    out = nc.dram_tensor("out", [N_GRAPHS, GH * 2], f32, kind="ExternalOutput")

    # half tables per layer (u8 rows of ROWB bytes)
    Ta = [nc.dram_tensor(f"T{l}a", [A_TOT, ROWB], u8, kind="Internal",
                         addr_space="Shared") for l in (1, 2)]
    Tb = [nc.dram_tensor(f"T{l}b", [B_TOT, ROWB], u8, kind="Internal",
                         addr_space="Shared") for l in (1, 2)]
    Tsha = [nc.dram_tensor(f"Tsh{l}a", [HA, ROWB], u8, kind="Internal")
            for l in (1, 2)]
    Tshb = [nc.dram_tensor(f"Tsh{l}b", [HB, ROWB], u8, kind="Internal")
            for l in (1, 2)]
    s_in = nc.dram_tensor("s_in", [N_GRAPHS, GH], f32, kind="Internal")
    s_out = nc.dram_tensor("s_out", [N_GRAPHS, GH], f32, kind="Internal",
                           addr_space="Shared")

    RG = [list(range(N_CORES))]

    with tile.TileContext(nc) as tc:
        with tc.tile_pool(name="const", bufs=1) as cp, \
             tc.tile_pool(name="own", bufs=1) as ownp, \
             tc.tile_pool(name="ga", bufs=2) as gap, \
             tc.tile_pool(name="gb", bufs=2) as gbp, \
             tc.tile_pool(name="pp", bufs=2) as ppool, \
             tc.tile_pool(name="ix", bufs=4) as ixp, \
             tc.tile_pool(name="sb", bufs=3) as sb, \
             tc.tile_pool(name="mlp", bufs=1) as mlppool, \
             tc.tile_pool(name="ps", bufs=2, space="PSUM") as ps, \
             tc.tile_pool(name="pse", bufs=2, space="PSUM") as pse, \
             tc.tile_pool(name="psacc", bufs=2, space="PSUM") as psacc, \
             tc.tile_pool(name="pspool", bufs=1, space="PSUM") as pspool:

            nc.gpsimd.load_library(library_config.mlp)

            # distinct gather sizes -> regs
            sizes = sorted({nc_ * 128 for _, nc_, _, _ in callsA} |
                           {nc_ * 128 for _, nc_, _, _ in callsB})
            nidx_regs = {n: nc.gpsimd.to_reg(n) for n in sizes}

            ident = cp.tile([128, 128], bf16)
            nc.sync.dma_start(ident[:], inp["ident"][:, :])

            wexp0 = cp.tile([18, MID], bf16, tag="wexp0")
            nc.sync.dma_start(wexp0[:], inp["Wexp0"][:, :])
            rhs_a, rhs_b = {}, {}
            for l, nma, nmb in [(1, "rhs1a", "rhs1b"), (2, "rhs2a", "rhs2b")]:
                wdt = inp[nma].shape[1]
                ta = cp.tile([128, wdt], bf16, tag=f"rhsa{l}")
                nc.sync.dma_start(ta[:], inp[nma][:, :])
                tb = cp.tile([1, wdt], bf16, tag=f"rhsb{l}")
                nc.sync.dma_start(tb[:], inp[nmb][:, :])
                rhs_a[l], rhs_b[l] = ta, tb
            bb_sb = []
            for l, nm in enumerate(["bb0", "bb1", "bb2"]):
                t = cp.tile(list(inp[nm].shape), bf16, tag=f"bb{l}")
                nc.sync.dma_start(t[:], inp[nm][:, :])
                bb_sb.append(t)
            bp_sb = cp.tile([128, NT, N_GRAPHS], bf16, tag="bpall")
            nc.sync.dma_start(bp_sb[:], inp["bpool"][:, :, :])

            xown = cp.tile([128, NT, 6], bf16, tag="xown")
            nc.sync.dma_start(xown[:], inp["x_own"][:, :, :])
            es0o = cp.tile([128, NT, 3], f32, tag="es0o")
            nc.sync.dma_start(es0o[:], inp["es0_own"][:, :, :])
            ed0o = cp.tile([128, NT, 3], bf16, tag="ed0o")
            nc.sync.dma_start(ed0o[:], inp["ed0_own"][:, :, :])

            # own-shard dequantized rows for self terms, layers 1,2
            own_deq = [ownp.tile([128, NT, MID if l == 1 else GH], bf16,
                                 tag=f"ownd{l}", name=f"ownd{l}")
                       for l in (1, 2)]
            # es/ed caches for self terms + ed broadcast, layers 1,2
            esed = {}
            for l, hh in ((1, 3), (2, 1)):
                esed[l] = (ownp.tile([128, NT, hh], bf16, tag=f"esc{l}",
                                     name=f"esc{l}"),
                           ownp.tile([128, NT, hh], bf16, tag=f"edc{l}",
                                     name=f"edc{l}"))

            # ---------------- loc MLP (independent; issue early) ------------
            locT = mlppool.tile([2, N_GRAPHS * N_LOCS], bf16)
            nc.sync.dma_start(locT[:], inp["locT"][:, :])
            wl1 = mlppool.tile([2, MLPH], bf16)
            nc.sync.dma_start(wl1[:], inp["wl1"][:, :])
            bl1c = mlppool.tile([128, 2], f32)
            nc.sync.dma_start(bl1c[:], inp["bl1c"][:, 0:1].rearrange(
                "(h p) o -> p (h o)", p=128))
            wl2a = mlppool.tile([128, GH], bf16)
            nc.sync.dma_start(wl2a[:], inp["wl2"][0:128, :])
            wl2b = mlppool.tile([128, GH], bf16)
            nc.sync.dma_start(wl2b[:], inp["wl2"][128:256, :])
            bl2c = mlppool.tile([GH, 1], f32)
            nc.sync.dma_start(bl2c[:], inp["bl2c"][:, :])
            identf = mlppool.tile([128, 128], f32)
            nc.vector.tensor_copy(identf[:], ident[:])
            mid_sb = [mlppool.tile([128, N_GRAPHS * N_LOCS], bf16,
                                   tag=f"mid{h}", name=f"mid{h}")
                      for h in range(2)]
            CH = 400
            nch = (N_GRAPHS * N_LOCS) // CH
            for h in range(2):
                for c in range(nch):
                    pm = ps.tile([128, CH], f32, space="PSUM", tag="pscr")
                    nc.tensor.matmul(pm[:], lhsT=wl1[:, h * 128:(h + 1) * 128],
                                     rhs=locT[:, c * CH:(c + 1) * CH],
                                     start=True, stop=True)
                    nc.scalar.activation(mid_sb[h][:, c * CH:(c + 1) * CH], pm[:],
                                         AF.Tanh, bias=bl1c[:, h:h + 1])
            lpT = mlppool.tile([128, N_GRAPHS], f32)
            for c in range(nch):
                po = ps.tile([128, CH], f32, space="PSUM", tag="pscr")
                nc.tensor.matmul(po[:], lhsT=wl2a[:],
                                 rhs=mid_sb[0][:, c * CH:(c + 1) * CH],
                                 start=True, stop=False)
                nc.tensor.matmul(po[:], lhsT=wl2b[:],
                                 rhs=mid_sb[1][:, c * CH:(c + 1) * CH],
                                 start=False, stop=True)
                ng = CH // N_LOCS
                nc.vector.reduce_sum(
                    lpT[:, c * ng:(c + 1) * ng],
                    po[:].rearrange("p (g l) -> p g l", l=N_LOCS),
                    axis=mybir.AxisListType.X)
            lpT2 = mlppool.tile([128, N_GRAPHS], f32)
            nc.vector.tensor_scalar(lpT2[:], lpT[:], 1.0 / N_LOCS, bl2c[:],
                                    AO.mult, AO.add)
            plp = ps.tile([N_GRAPHS, 128], f32, space="PSUM", tag="pscr")
            nc.tensor.matmul(plp[:], lhsT=lpT2[:], rhs=identf[:],
                             start=True, stop=True)
            out_sb = mlppool.tile([N_GRAPHS, 2 * GH], f32)
            nc.vector.tensor_copy(out_sb[:, GH:], plp[:])

            psum_S = pspool.tile([N_GRAPHS, GH], f32, space="PSUM")

            # ---- shared epilogue helper: table build from hin [128, F] ----
            def build_table(l, t, hin, Fin):
                """hin [128, Fin] bf16 -> quantized row of T_{l} for tile t."""
                ph1 = ps.tile([128, 128], f32, space="PSUM", tag="pscr")
                nc.tensor.matmul(ph1[:], lhsT=hin[:, 0:128], rhs=ident[:],
                                 start=True, stop=True)
                hTa = sb.tile([128, 128], bf16, tag="hTa")
                nc.vector.tensor_copy(hTa[:], ph1[:])
                ph2 = ps.tile([1, 128], f32, space="PSUM", tag="pscr")
                nc.tensor.matmul(ph2[:], lhsT=hin[:, 128:Fin], rhs=ident[:],
                                 start=True, stop=True)
                hTb = sb.tile([1, 128], bf16, tag="hTb")
                nc.vector.tensor_copy(hTb[:], ph2[:])
                F2 = MID if l == 1 else GH      # next table's feature width
                H2 = 3 if l == 1 else 1
                W2_ = F2 + 2 * H2
                ptab = ps.tile([128, W2_], f32, space="PSUM", tag="pscr")
                nc.tensor.matmul(ptab[:], lhsT=hTa[:], rhs=rhs_a[l][:, :W2_],
                                 start=True, stop=False)
                nc.tensor.matmul(ptab[:], lhsT=hTb[:], rhs=rhs_b[l][:, :W2_],
                                 start=False, stop=True)
                # quantize h' -> int8 with fp16 scale; cache es/ed
                am = sb.tile([128, 1], f32, tag="am")
                nc.vector.reduce_max(am[:], ptab[:, 0:F2],
                                     axis=mybir.AxisListType.X,
                                     apply_absolute_value=True)
                sc = sb.tile([128, 1], f32, tag="sc")
                nc.vector.tensor_scalar(sc[:], am[:], 1.0 / 127.0, 1e-20,
                                        AO.mult, AO.max)
                sci = sb.tile([128, 1], f32, tag="sci")
                nc.vector.reciprocal(sci[:], sc[:])
                nc.vector.tensor_copy(own_deq[l - 1][:, t, :], ptab[:, 0:F2])
                stg = sb.tile([128, ROWB], u8, tag="stg")
                nc.vector.memset(stg[:, 137:ROWB], 0.0)
                nc.vector.memset(stg[:, 4:8], 0.0)
                nc.vector.tensor_copy(stg[:, 0:2].bitcast(fp16), sc[:])
                nc.vector.tensor_copy(stg[:, 2:2 + 2 * H2].bitcast(fp16),
                                      ptab[:, F2:F2 + H2])
                nc.scalar.activation(stg[:, 8:8 + F2].bitcast(i8),
                                     ptab[:, 0:F2], AF.Copy, scale=sci[:])
                if F2 < MID:
                    nc.vector.memset(stg[:, 8 + F2:137], 0.0)
                esc, edc = esed[l]
                nc.vector.tensor_copy(esc[:, t, :], ptab[:, F2:F2 + H2])
                nc.vector.tensor_copy(edc[:, t, :], ptab[:, F2 + H2:F2 + 2 * H2])
                # ship to shard table region
                if t < NT_A:
                    nc.sync.dma_start(Tsha[l - 1][t * 128:(t + 1) * 128, :], stg[:])
                else:
                    tt = t - NT_A
                    nc.sync.dma_start(Tshb[l - 1][tt * 128:(tt + 1) * 128, :], stg[:])

            # ================= layer 0 (no gather) =================
            for t in range(NT):
                KAt, KBt, Kt = KA[t], KB[t], KA[t] + KB[t]
                f0 = offT[t]
                pd = ppool.tile([128, MAXK * 128], fp8, tag="pd")
                nc.sync.dma_start(pd[:, 0:Kt * 128],
                                  inp["pdst"][:, f0 * 128:(f0 + Kt) * 128])
                pe_ = ppool.tile([128, MAXK * 128], fp8, tag="pe")
                nc.scalar.dma_start(pe_[:, 0:Kt * 128],
                                    inp["pedge"][:, f0 * 128:(f0 + Kt) * 128])
                xet = sb.tile([128, MAXK, 12], bf16, tag="xet")
                nc.sync.dma_start(xet[:, 0:Kt, :], inp["xe"][:, f0:f0 + Kt, :])
                # ed broadcast
                pede = pse.tile([128, MAXK, 3], f32, space="PSUM",
                                tag="pede")
                for j in range(Kt):
                    nc.tensor.matmul(pede[:, j, :],
                                     lhsT=pd[:, j * 128:(j + 1) * 128],
                                     rhs=ed0o[:, t, :], start=True, stop=True)
                esum = sb.tile([128, MAXK, 3], f32, tag="esum")
                nc.vector.tensor_tensor(out=esum[:, 0:Kt, :],
                                        in0=xet[:, 0:Kt, 6:9],
                                        in1=pede[:, 0:Kt, :], op=AO.add)
                lk = sb.tile([128, MAXK, 3], f32, tag="lk")
                nc.scalar.activation(lk[:, 0:Kt, :], esum[:, 0:Kt, :],
                                     AF.Prelu, alpha=NEG)
                w = sb.tile([128, MAXK, 3], bf16, tag="w")
                nc.scalar.activation(w[:, 0:Kt, :], lk[:, 0:Kt, :], AF.Exp)
                rt = sb.tile([128, MAXK, 21], bf16, tag="rt")
                for h in range(3):
                    nc.vector.tensor_tensor(
                        out=rt[:, 0:Kt, h * 6:(h + 1) * 6],
                        in0=xet[:, 0:Kt, 0:6],
                        in1=w[:, 0:Kt, h:h + 1].to_broadcast([128, Kt, 6]),
                        op=AO.mult)
                acc = psacc.tile([128, 21], f32, space="PSUM", tag="acc")
                for j in range(Kt):
                    nc.tensor.matmul(acc[:, 0:18],
                                     lhsT=pe_[:, j * 128:(j + 1) * 128],
                                     rhs=rt[:, j, 0:18],
                                     start=(j == 0), stop=(j == Kt - 1))
                    nc.tensor.matmul(acc[:, 18:21],
                                     lhsT=pe_[:, j * 128:(j + 1) * 128],
                                     rhs=w[:, j, :],
                                     start=(j == 0), stop=(j == Kt - 1))
                # self-loop term + normalize
                se = sb.tile([128, 3], f32, tag="se")
                nc.vector.tensor_tensor(out=se[:], in0=es0o[:, t, :],
                                        in1=ed0o[:, t, :], op=AO.add)
                slk = sb.tile([128, 3], f32, tag="slk")
                nc.scalar.activation(slk[:], se[:], AF.Prelu, alpha=NEG)
                selfm = sb.tile([128, 21], f32, tag="selfm")
                nc.scalar.activation(selfm[:, 18:21], slk[:], AF.Exp)
                for h in range(3):
                    nc.vector.tensor_tensor(
                        out=selfm[:, h * 6:(h + 1) * 6],
                        in0=xown[:, t, :],
                        in1=selfm[:, 18 + h:19 + h].to_broadcast([128, 6]),
                        op=AO.mult)
                accs = sb.tile([128, 21], f32, tag="accs")
                nc.vector.tensor_tensor(out=accs[:], in0=acc[:], in1=selfm[:],
                                        op=AO.add)
                zr = sb.tile([128, 3], f32, tag="zr")
                nc.vector.reciprocal(zr[:], accs[:, 18:21])
                un = sb.tile([128, 18], bf16, tag="un")
                for h in range(3):
                    nc.scalar.activation(un[:, h * 6:(h + 1) * 6],
                                         accs[:, h * 6:(h + 1) * 6],
                                         AF.Copy, scale=zr[:, h:h + 1])
                pt_ = ps.tile([18, 128], f32, space="PSUM", tag="pscr")
                nc.tensor.matmul(pt_[:], lhsT=un[:], rhs=ident[:],
                                 start=True, stop=True)
                accT = sb.tile([18, 128], bf16, tag="accT")
                nc.vector.tensor_copy(accT[:], pt_[:])
                h0ps = ps.tile([128, MID], f32, space="PSUM", tag="pscr")
                nc.tensor.matmul(h0ps[:], lhsT=accT[:], rhs=wexp0[:],
                                 start=True, stop=True)
                hin = sb.tile([128, MID], bf16, tag="hin")
                nc.vector.tensor_add(hin[:], h0ps[:], bb_sb[0][:, :MID])
                nc.scalar.activation(hin[:], hin[:], AF.Prelu, alpha=NEG)
                build_table(1, t, hin, MID)
                if t == NT_A - 1:
                    nc.gpsimd.collective_compute(
                        "AllGather", AO.bypass, replica_groups=RG,
                        ins=[Tsha[0][:, :]], outs=[Ta[0][:, :]])
            nc.gpsimd.collective_compute(
                "AllGather", AO.bypass, replica_groups=RG,
                ins=[Tshb[0][:, :]], outs=[Tb[0][:, :]])

            # ================= layers 1, 2 (gathered) =================
            for l in (1, 2):
                F = MID if l == 1 else GH
                Hh = 3 if l == 1 else 1
                O = F // Hh
                RW = F + Hh
                tabA = Ta[l - 1]
                tabB = Tb[l - 1]
                esc, edc = esed[l]
                ownl = own_deq[l - 1]

                gbufA = [gap.tile([128, CALL_COLS, ROWB], u8,
                                  tag=f"gA{i}", name=f"gA{i}_{l}")
                         for i in range(2)]
                gbufB = [gbp.tile([128, CALL_COLS, ROWB], u8,
                                  tag=f"gB{i}", name=f"gB{i}_{l}")
                         for i in range(2)]
                issuedA = [False] * len(callsA)
                issuedB = [False] * len(callsB)

                def issue(calls, issued, idx_inp, tab, gbufs, i):
                    if issued[i]:
                        return
                    issued[i] = True
                    c0, ncol, _, _ = calls[i]
                    n = ncol * 128
                    # load idx slice (transient)
                    it = ixp.tile([128, CALL_COLS * 8], i16, tag="idxt")
                    nc.sync.dma_start(it[:, 0:ncol * 8],
                                      idx_inp[:, c0 * 8:(c0 + ncol) * 8])
                    # single_packet=False: the SDMA packet ceiling is <=64
                    # descriptors; a 36-col call emits ~290 descs per engine.
                    nc.gpsimd.dma_gather(
                        out_ap=gbufs[i % 2][:, 0:ncol, :], in_ap=tab[:, :],
                        idxs_ap=it[:, 0:ncol * 8], num_idxs=n,
                        num_idxs_reg=nidx_regs[n], elem_size=ROWB,
                        single_packet=False)

                for t in range(NT):
                    KAt, KBt, Kt = KA[t], KB[t], KA[t] + KB[t]
                    ia, ib = tcallA[t], tcallB[t]
                    issue(callsA, issuedA, inp["idxA"], tabA, gbufA, ia)
                    issue(callsB, issuedB, inp["idxB"], tabB, gbufB, ib)
                    # also prefetch next calls
                    if ia + 1 < len(callsA):
                        issue(callsA, issuedA, inp["idxA"], tabA, gbufA, ia + 1)
                    if ib + 1 < len(callsB):
                        issue(callsB, issuedB, inp["idxB"], tabB, gbufB, ib + 1)
                    gA = gbufA[ia % 2][:, offA[t] - callsA[ia][0]:
                                       offA[t] - callsA[ia][0] + KAt, :]
                    gB = gbufB[ib % 2][:, offB[t] - callsB[ib][0]:
                                       offB[t] - callsB[ib][0] + KBt, :]

                    f0 = offT[t]
                    pd = ppool.tile([128, MAXK * 128], fp8, tag="pd")
                    nc.sync.dma_start(pd[:, 0:Kt * 128],
                                      inp["pdst"][:, f0 * 128:(f0 + Kt) * 128])
                    pe_ = ppool.tile([128, MAXK * 128], fp8, tag="pe")
                    nc.scalar.dma_start(pe_[:, 0:Kt * 128],
                                        inp["pedge"][:, f0 * 128:(f0 + Kt) * 128])
                    pede = pse.tile([128, MAXK, 3], f32, space="PSUM",
                                    tag="pede")
                    for j in range(Kt):
                        nc.tensor.matmul(pede[:, j, :Hh],
                                         lhsT=pd[:, j * 128:(j + 1) * 128],
                                         rhs=edc[:, t, :], start=True, stop=True)
                    # w per piece (A then B)
                    esum = sb.tile([128, MAXK, 3], f32, tag="esum")
                    wt = sb.tile([128, MAXK, 3], bf16, tag="w")
                    wsc = sb.tile([128, MAXK, 3], bf16, tag="wsc")
                    rt = sb.tile([128, MAXK, MID + 3], bf16, tag="rt")
                    for g, ko, kn in ((gA, 0, KAt), (gB, KAt, KBt)):
                        nc.vector.tensor_tensor(
                            out=esum[:, ko:ko + kn, :Hh],
                            in0=g[:, :, 2:2 + 2 * Hh].bitcast(fp16),
                            in1=pede[:, ko:ko + kn, :Hh], op=AO.add)
                        nc.scalar.activation(esum[:, ko:ko + kn, :Hh],
                                             esum[:, ko:ko + kn, :Hh],
                                             AF.Prelu, alpha=NEG)
                        nc.scalar.activation(wt[:, ko:ko + kn, :Hh],
                                             esum[:, ko:ko + kn, :Hh], AF.Exp)
                        nc.vector.tensor_tensor(
                            out=wsc[:, ko:ko + kn, :Hh],
                            in0=wt[:, ko:ko + kn, :Hh],
                            in1=g[:, :, 0:2].bitcast(fp16).to_broadcast(
                                [128, kn, Hh]),
                            op=AO.mult)
                        for h in range(Hh):
                            nc.vector.tensor_tensor(
                                out=rt[:, ko:ko + kn, h * O:(h + 1) * O],
                                in0=g[:, :, 8 + h * O:8 + (h + 1) * O].bitcast(i8),
                                in1=wsc[:, ko:ko + kn, h:h + 1].to_broadcast(
                                    [128, kn, O]),
                                op=AO.mult)
                    acc = psacc.tile([128, MID + 3], f32, space="PSUM", tag="acc")
                    for j in range(Kt):
                        nc.tensor.matmul(acc[:, 0:F],
                                         lhsT=pe_[:, j * 128:(j + 1) * 128],
                                         rhs=rt[:, j, 0:F],
                                         start=(j == 0), stop=(j == Kt - 1))
                        nc.tensor.matmul(acc[:, F:F + Hh],
                                         lhsT=pe_[:, j * 128:(j + 1) * 128],
                                         rhs=wt[:, j, :Hh],
                                         start=(j == 0), stop=(j == Kt - 1))
                    # self-loop + normalize
                    se = sb.tile([128, 3], f32, tag="se")
                    nc.vector.tensor_tensor(out=se[:, :Hh], in0=esc[:, t, :],
                                            in1=edc[:, t, :], op=AO.add)
                    nc.scalar.activation(se[:, :Hh], se[:, :Hh],
                                         AF.Prelu, alpha=NEG)
                    selfm = sb.tile([128, MID + 3], f32, tag="selfm")
                    nc.scalar.activation(selfm[:, F:F + Hh], se[:, :Hh], AF.Exp)
                    for h in range(Hh):
                        nc.vector.tensor_tensor(
                            out=selfm[:, h * O:(h + 1) * O],
                            in0=ownl[:, t, h * O:(h + 1) * O],
                            in1=selfm[:, F + h:F + h + 1].to_broadcast([128, O]),
                            op=AO.mult)
                    accs = sb.tile([128, MID + 3], f32, tag="accs")
                    nc.vector.tensor_tensor(out=accs[:, :RW], in0=acc[:, :RW],
                                            in1=selfm[:, :RW], op=AO.add)
                    zr = sb.tile([128, 3], f32, tag="zr")
                    nc.vector.reciprocal(zr[:, :Hh], accs[:, F:F + Hh])
                    ob = sb.tile([128, MID], bf16, tag="ob")
                    for h in range(Hh):
                        nc.scalar.activation(ob[:, h * O:(h + 1) * O],
                                             accs[:, h * O:(h + 1) * O],
                                             AF.Copy, scale=zr[:, h:h + 1])
                    if l == 1:
                        hin = sb.tile([128, MID], bf16, tag="hin")
                        nc.vector.tensor_add(hin[:], ob[:], bb_sb[1][:, :MID])
                        nc.scalar.activation(hin[:], hin[:], AF.Prelu, alpha=NEG)
                        build_table(2, t, hin, MID)
                        if t == NT_A - 1:
                            nc.gpsimd.collective_compute(
                                "AllGather", AO.bypass, replica_groups=RG,
                                ins=[Tsha[1][:, :]], outs=[Ta[1][:, :]])
                    else:
                        ob2 = sb.tile([128, GH], bf16, tag="ob2")
                        nc.vector.tensor_add(ob2[:], ob[:, :GH], bb_sb[2][:, :GH])
                        nc.tensor.matmul(psum_S[:], lhsT=bp_sb[:, t, :],
                                         rhs=ob2[:],
                                         start=(t == 0), stop=(t == NT - 1))
                if l == 1:
                    nc.gpsimd.collective_compute(
                        "AllGather", AO.bypass, replica_groups=RG,
                        ins=[Tshb[1][:, :]], outs=[Tb[1][:, :]])

            # pooling: AllReduce of per-shard sums, then divide by counts
            ssb = sb.tile([N_GRAPHS, GH], f32, tag="ssb")
            nc.vector.tensor_copy(ssb[:], psum_S[:])
            nc.sync.dma_start(s_in[:, :], ssb[:])
            nc.gpsimd.collective_compute(
                "AllReduce", AO.add, replica_groups=RG,
                ins=[s_in[:, :]], outs=[s_out[:, :]])
            sfull = sb.tile([N_GRAPHS, GH], f32, tag="sfull")
            nc.sync.dma_start(sfull[:], s_out[:, :])
            civ = sb.tile([N_GRAPHS, 1], f32, tag="civ")
            nc.sync.dma_start(civ[:], inp["cntinv"][:, :])
            nc.vector.tensor_scalar_mul(out_sb[:, 0:GH], sfull[:], civ[:])
            nc.sync.dma_start(out[:, :], out_sb[:])

    lower_extended_insts(nc)
    return nc


def kernel(**inputs):
    key = "k"
    in_maps, meta = _host_prep(**inputs)
    if key not in _CACHE or _CACHE[key][1] != meta:
        nc = _build(meta)
        _CACHE[key] = (nc, meta)
    nc = _CACHE[key][0]
    from concourse.bass_utils import run_bass_kernel_spmd
    res = run_bass_kernel_spmd(nc, in_maps, core_ids=list(range(N_CORES)))
    return np.asarray(res.results[0]["out"])
